# revision 51
# baseline (speedup 1.0000x reference)
"""Trainium2 Bass kernel for nn_DifferentPooling (GNN message passing).

Strategy (8 NeuronCores, SPMD):
  - Nodes padded to NP = 8*CHUNK, partitioned by node id across cores in a
    K=4 chunk-permuted address space; edges partitioned by dst core and
    bucketed into 128-node dst windows. Aggregations (segment sum /
    softmax-sum) run on the TensorEngine as one-hot matmuls; the one-hot
    S_en / S_en^T matrices are built host-side and streamed from DRAM.
  - Per layer the 8 per-core feature chunks are AllGather'd (4 chunks,
    issued as producing windows finish). The GAT-feeding tables travel as
    fp8-e4m3 (half collective bytes on the serial collective device) and
    are upcast to bf16 gather tables on the consumer in partition-major
    stripes (DVE+Act split). The GC2 table is precision-critical and stays
    bf16.
  - Layers (except GC1, whose table is the kernel input) process all
    windows in two passes: lo-half edge tiles first (only needs the lo
    half of the incoming table, i.e. the first 2 AllGather chunks), then
    hi-half tiles + combine with the saved lo partials - overlapping each
    layer's start with its predecessor's trailing collectives.
  - GATv2: eps/leaky-relu computed feature-major (weight-stationary
    matmuls, N=512), logits via a block-diagonal attention matmul
    (edge-major out), exp on Act; alpha-weighting and the [num | den]
    aggregation stay edge-major with den in 8 extra PSUM columns.
  - Softmax uses exp(logit) without max subtraction (logits tiny) with a
    1e-30 guard; graph max-pooling via masked-max segments and a small
    AllGather; replicated fp32 MLP head.

All biases in this problem are zeros by spec (fill="zeros"); they are not
applied on device.
"""

import sys

sys.path.insert(0, "/opt/trn_rl_repo")

import numpy as np
import ml_dtypes

bf16 = ml_dtypes.bfloat16
fp8 = ml_dtypes.float8_e4m3fn

N_CORES = 8
P = 128  # window size / partition count
N_REAL = 50000
E_REAL = 500000
G = 64
HID = 128
HEADS = 8
DH = 16
OUT = 256


# ---------------------------------------------------------------------------
# Host-side preprocessing
# ---------------------------------------------------------------------------

def _wrap_idx(arr):
    """int idx array (len % 16 == 0) -> [128, len/16] int16 wrapped layout:
    idx i lives at [i % 16, i // 16], replicated across the 8 groups of 16
    partitions (one per Q7 core)."""
    a = np.asarray(arr, np.int16).reshape(-1, 16).T  # [16, cols]
    return np.tile(a, (8, 1))  # [128, cols]


def prep(src, dst, node2graph, nw_per_core, kchunks=4):
    """Build per-core edge/window metadata. Returns (cfg, host arrays).

    The node table on device lives in a chunk-permuted layout so each
    layer's AllGather can be split into `kchunks` window-chunks issued as
    soon as the producing windows finish: address order is
    [chunk][rank][window-in-chunk][pos], matching what chunked AllGathers
    over agin row-slices naturally produce. All gather indices below are
    built in permuted address space.
    """
    NW = nw_per_core
    CHUNK = NW * P
    NP = N_CORES * CHUNK
    HALF = NP // 2
    N = len(node2graph)
    E = len(src)

    src = np.asarray(src, np.int64)
    dst = np.asarray(dst, np.int64)
    n2g = np.asarray(node2graph, np.int64)

    # chunk structure (in windows): even split measured best (smaller-first
    # and K=5 both regressed against the cost model's 15us/chunk overhead).
    K = kchunks
    wc = [NW // K] * K
    for c in range(NW - sum(wc)):
        wc[c] += 1
    w0s = np.cumsum([0] + wc)[:-1]
    chunk_of_win = np.repeat(np.arange(K), wc)
    cbase = N_CORES * P * np.cumsum([0] + wc)[:-1]
    # lo/hi gather split at the chunk K/2 boundary so the lo and hi table
    # halves can live in separate DRAM tensors (precise collective deps)
    SPLIT = int(cbase[K // 2]) if K >= 2 else HALF

    def paddr(n):
        r, local = np.divmod(np.asarray(n, np.int64), CHUNK)
        w, p = np.divmod(local, P)
        c = chunk_of_win[w]
        return cbase[c] + (r * np.asarray(wc)[c] + (w - w0s[c])) * P + p

    outdeg = np.zeros(NP, np.float32)
    np.add.at(outdeg, src, 1.0)
    indeg = np.zeros(NP, np.float32)
    np.add.at(indeg, dst, 1.0)
    ns = np.maximum(outdeg, 1.0) ** -0.5
    nd = np.maximum(indeg, 1.0) ** -0.5

    # sort edges by dst, bucket into windows; src ids move to permuted space
    order = np.argsort(dst, kind="stable")
    sdst = dst[order]
    ssrc = paddr(src[order])
    n_win_total = NP // P
    win_starts = np.searchsorted(sdst, np.arange(0, NP + 1, P))

    # per (global window): lo/hi edge lists sorted by src
    lo_lists, hi_lists = [], []
    max_lo = max_hi = 1
    for w in range(n_win_total):
        a, b = win_starts[w], win_starts[w + 1]
        es, ed = ssrc[a:b], sdst[a:b] - w * P
        m = es < SPLIT
        ordl = np.argsort(es[m], kind="stable")
        ordh = np.argsort(es[~m], kind="stable")
        lo_lists.append((es[m][ordl], ed[m][ordl]))
        hi_lists.append((es[~m][ordh] - SPLIT, ed[~m][ordh]))
        max_lo = max(max_lo, len(lo_lists[-1][0]))
        max_hi = max(max_hi, len(hi_lists[-1][0]))

    L = (max_lo + P - 1) // P
    H = (max_hi + P - 1) // P
    T = L + H

    # spans of SPAN_W windows (gather batching granularity)
    SPAN_W = 8 if NW >= 8 else 2
    spans = []
    w0 = 0
    while w0 < NW:
        spans.append((w0, min(SPAN_W, NW - w0)))
        w0 += SPAN_W

    per_core = []
    for c in range(N_CORES):
        idx_lo = np.zeros((NW, L * P), np.int64)
        dst_lo = np.full((NW, L * P), P, np.int64)  # sentinel 128
        idx_hi = np.zeros((NW, H * P), np.int64)
        dst_hi = np.full((NW, H * P), P, np.int64)
        for w in range(NW):
            el, dl = lo_lists[c * NW + w]
            eh, dh_ = hi_lists[c * NW + w]
            idx_lo[w, : len(el)] = el
            dst_lo[w, : len(dl)] = dl
            idx_hi[w, : len(eh)] = eh
            dst_hi[w, : len(dh_)] = dh_
        # dstloc: [NW*T, 128] -> transpose to [128, NW*T]; col w*T+t
        dstloc = np.concatenate(
            [dst_lo.reshape(NW, L, P), dst_hi.reshape(NW, H, P)], axis=1
        ).reshape(NW * T, P)
        ndw = nd[c * CHUNK : (c + 1) * CHUNK].reshape(NW, P).T.copy()
        nsw = ns[c * CHUNK : (c + 1) * CHUNK].reshape(NW, P).T.copy()
        # host-built one-hot selection matrices:
        # sden[w, e, t*128+n] = (dst-pos of edge slot (w,t,e) == n)
        # snden[w, p, t*128+e] = sden[w, e, t*128+p]  (transpose)
        eye = np.zeros((P + 1, P), bf16)
        eye[np.arange(P), np.arange(P)] = 1
        oh = eye[dstloc]                       # [NW*T, P(e), P(n)]
        sden_h = np.ascontiguousarray(
            oh.reshape(NW, T, P, P).transpose(0, 2, 1, 3)
            .reshape(NW, P, T * P))
        snden_h = np.ascontiguousarray(
            oh.reshape(NW, T, P, P).transpose(0, 3, 1, 2)
            .reshape(NW, P, T * P))
        per_core.append(
            dict(
                idx_lo=_wrap_idx(idx_lo.reshape(-1)),
                idx_hi=_wrap_idx(idx_hi.reshape(-1)),
                sden=sden_h,
                sden8=sden_h.astype(fp8),
                snden=snden_h.astype(fp8),
                ndw=np.ascontiguousarray(ndw, np.float32),
                nsw=np.ascontiguousarray(nsw, np.float32),
            )
        )

    # pooling segments per core: runs of equal graph id inside each window
    n2g_pad = np.full(NP, -1, np.int64)
    n2g_pad[:N] = n2g
    seg_all = []  # per core: list of (w, j0, j1, g)
    KSEG = 1
    for c in range(N_CORES):
        segs = []
        for w in range(NW):
            ids = n2g_pad[c * CHUNK + w * P : c * CHUNK + (w + 1) * P]
            j = 0
            wsegs = []
            while j < P:
                g = ids[j]
                k = j
                while k < P and ids[k] == g:
                    k += 1
                if g >= 0:
                    wsegs.append((j, k, int(g)))
                j = k
            KSEG = max(KSEG, len(wsegs))
            segs.append(wsegs)
        seg_all.append(segs)

    BIG = np.float32(1e30)
    NSEG = NW * KSEG
    for c in range(N_CORES):
        maskvec = np.full((NW, KSEG, P), -BIG, np.float32)
        gmask = np.full((G, NSEG), -BIG, np.float32)
        for w in range(NW):
            for k, (j0, j1, g) in enumerate(seg_all[c][w]):
                maskvec[w, k, j0:j1] = BIG
                gmask[g, w * KSEG + k] = BIG
        per_core[c]["poolmask"] = maskvec.reshape(NW, KSEG * P).astype(bf16)
        per_core[c]["gmask"] = gmask.astype(bf16)

    cfg = dict(NW=NW, CHUNK=CHUNK, NP=NP, HALF=HALF, SPLIT=SPLIT,
               L=L, H=H, T=T, spans=spans, KSEG=KSEG,
               chunks=list(zip(w0s.tolist(), wc)))
    perm_src = np.empty(NP, np.int64)  # perm_src[paddr] = original id
    perm_src[paddr(np.arange(NP))] = np.arange(NP)
    return cfg, per_core, ns, nd, perm_src


# ---------------------------------------------------------------------------
# Bass kernel builder
# ---------------------------------------------------------------------------

def build_nc(cfg):
    import concourse.bacc as bacc
    import concourse.bass as bass
    import concourse.mybir as mybir
    import concourse.tile as tile
    from concourse.masks import make_identity

    NW, CHUNK, NP, HALF = cfg["NW"], cfg["CHUNK"], cfg["NP"], cfg["HALF"]
    L, H, T, spans, KSEG = cfg["L"], cfg["H"], cfg["T"], cfg["spans"], cfg["KSEG"]
    chunks = cfg["chunks"]
    SPLIT = cfg["SPLIT"]
    FP = mybir.dt.float32
    BF = mybir.dt.bfloat16
    F8 = mybir.dt.float8e4
    AO = mybir.AluOpType
    AFT = mybir.ActivationFunctionType

    nc = bacc.Bacc("TRN2", target_bir_lowering=False, debug=False,
                   num_devices=N_CORES)

    def din(name, shape, dt=BF):
        return nc.dram_tensor(name, shape, dt, kind="ExternalInput")

    table0 = (din("table0lo", [SPLIT, P]), din("table0hi", [NP - SPLIT, P]))
    Wgc = [din(f"Wgc{i}", [P, P]) for i in range(2)]
    Ws = [din(f"Ws{i}", [P, P]) for i in range(3)]
    Wd = [din(f"Wd{i}", [P, P]) for i in range(3)]
    abd = [din(f"abd{i}", [P, HEADS]) for i in range(3)]
    Wc1 = din("Wc1", [P, P], FP)
    Wc2 = din("Wc2", [P, 64], FP)
    Wc3 = din("Wc3", [64, OUT], FP)
    idx_lo = din("idx_lo", [P, NW * L * P // 16], mybir.dt.int16)
    idx_hi = din("idx_hi", [P, NW * H * P // 16], mybir.dt.int16)
    sden = din("sden", [NW, P, T * P])
    sden8 = din("sden8", [NW, P, T * P], F8)
    snden = din("snden", [NW, P, T * P], F8)
    ndw = din("ndw", [P, NW], FP)
    nsw = din("nsw", [P, NW], FP)
    poolmask = din("poolmask", [NW, KSEG * P])
    gmask = din("gmask", [G, NW * KSEG])

    out_ext = nc.dram_tensor("out", [G, OUT], FP, kind="ExternalOutput")

    # internal DRAM
    # agin: bf16 own-chunk copies (hch source); aginq: fp8 collective inputs
    # for the GAT-feeding layers (tables 2..4 travel as fp8 and are upcast
    # to bf16 gather tables on the consumer side).
    agin = [nc.dram_tensor("agin0", [CHUNK, P], BF)]
    aginq = [nc.dram_tensor(f"aginq{i}", [CHUNK, P], F8) for i in range(4)]
    tables = [table0,
              (nc.dram_tensor("tlo1", [SPLIT, P], BF, addr_space="Shared"),
               nc.dram_tensor("thi1", [NP - SPLIT, P], BF,
                              addr_space="Shared"))]
    rc = [wcc * P * N_CORES for (_w0c, wcc) in chunks]
    tables8 = [None, None]
    for i in range(2, 5):
        tables8.append(tuple(
            nc.dram_tensor(f"t8c{i}_{k}", [rc[k], P], F8,
                           addr_space="Shared")
            for k in range(len(chunks))))
        tables.append(
            (nc.dram_tensor(f"tlo{i}", [SPLIT, P], BF),
             nc.dram_tensor(f"thi{i}", [NP - SPLIT, P], BF)))
    hgpart = nc.dram_tensor("hgpart", [P, G], FP)
    hgall = nc.dram_tensor("hgall", [N_CORES * P, G], FP, addr_space="Shared")

    RG = [list(range(N_CORES))]

    with tile.TileContext(nc) as tc:
        import contextlib

        ctx = contextlib.ExitStack()
        with ctx:
            const_pool = ctx.enter_context(tc.tile_pool(name="const", bufs=1))
            stg_pool = ctx.enter_context(tc.tile_pool(name="stg", bufs=2))
            sb_pool = ctx.enter_context(tc.tile_pool(name="sb", bufs=3))
            chunk_pool = ctx.enter_context(tc.tile_pool(name="chunk", bufs=1))
            ps_pool = ctx.enter_context(
                tc.tile_pool(name="ps", bufs=2, space="PSUM")
            )
            snt_pool = ctx.enter_context(
                tc.tile_pool(name="snt", bufs=1, space="PSUM")
            )
            agg_pool = ctx.enter_context(
                tc.tile_pool(name="agg", bufs=2, space="PSUM")
            )
            mini_ps = ctx.enter_context(
                tc.tile_pool(name="minips", bufs=2, space="PSUM")
            )

            # --- constants in SBUF ---
            ident_bf = const_pool.tile([P, P], BF, tag="identbf")
            make_identity(nc, ident_bf[:])
            ident_f = const_pool.tile([P, P], FP, tag="identf")
            make_identity(nc, ident_f[:])

            def load_const(h, shape, dt=BF, tag=None):
                t = const_pool.tile(shape, dt, tag=tag or h.name)
                nc.sync.dma_start(t[:], h[:])
                return t

            Wgc_sb = [load_const(w, [P, P]) for w in Wgc]
            Ws_sb = [load_const(w, [P, P]) for w in Ws]
            Wd_sb = [load_const(w, [P, P]) for w in Wd]
            abd_sb = [load_const(w, [P, HEADS]) for w in abd]
            ndw_sb = load_const(ndw, [P, NW], FP)
            nsw_sb = load_const(nsw, [P, NW], FP)
            idxlo_sb = load_const(idx_lo, [P, NW * L * P // 16], mybir.dt.int16)
            idxhi_sb = load_const(idx_hi, [P, NW * H * P // 16], mybir.dt.int16)

            SLOAD_W = 4  # windows per S_en reload DMA

            def s_en_load(w0, nwin, t0, tn, src_t=None, dt=None, tag="sload"):
                """Load S_en tiles [t0, t0+tn) for windows [w0, w0+nwin)."""
                if src_t is None:
                    src_t, dt = sden, BF
                sload = sb_pool.tile([P, SLOAD_W, max(L, H) * P], dt,
                                     tag=tag, bufs=2)
                nc.sync.dma_start(
                    sload[:, :nwin, : tn * P],
                    src_t[w0 : w0 + nwin, :,
                          t0 * P : (t0 + tn) * P].rearrange("w p f -> p w f"),
                )
                return sload

            def gather_span(table_l, w0, nw, transpose, which):
                """Gather the lo or hi edges of windows [w0, w0+nw).
                transpose -> [128, 1, n] column tiles, else
                [128, ntiles, 128] row tiles."""
                if which == "lo":
                    n, idx_sb, colpos = nw * L * P, idxlo_sb, w0 * L * P
                    half = table_l[0][0:SPLIT, :]
                else:
                    n, idx_sb, colpos = nw * H * P, idxhi_sb, w0 * H * P
                    half = table_l[1][0 : NP - SPLIT, :]
                nmax = nw * max(L, H) * P
                if transpose:
                    t = stg_pool.tile([P, 1, nmax], BF, tag="stg", bufs=3)
                    t = t[:, :, :n]
                else:
                    t = stg_pool.tile([P, nmax // P, P], BF, tag="stg", bufs=3)
                    t = t[:, : n // P, :]
                nc.gpsimd.dma_gather(
                    t[:, :, :],
                    half,
                    idx_sb[:, colpos // 16 : (colpos + n) // 16],
                    n,
                    n,
                    P,
                    transpose=transpose,
                    single_packet=False,
                )
                return t

            chunk_end = {w0c + wcc - 1: (w0c, wcc) for (w0c, wcc) in chunks}

            def flush_chunk(w, hnew, agin_out, table_out, aginq_out=None,
                            table8_out=None, h8ch=None):
                """After window w completes, DMA the finished chunk's rows
                out (bf16 agin copy and/or fp8 aginq). The AllGather itself
                is emitted a few windows LATER (see drain_ags): collectives
                issue from the Pool sequencer, and a collective emitted
                right at chunk completion holds Pool.SEQ waiting for the
                chunk DMA - blocking the remaining span gathers behind it.
                Delaying emission lets it issue with its input already in
                DRAM (short SEQ hold) while still starting early."""
                if w not in chunk_end:
                    return
                if aginq_out is None and agin_out is None:
                    return
                w0c, wcc = chunk_end[w]
                r0, r1 = w0c * P, (w0c + wcc) * P
                g0 = r0 * N_CORES
                kc = [i for i, (a, _b) in enumerate(chunks) if a == w0c][0]
                emit_w = min(w + 3, NW - 1)
                if table8_out is not None:
                    nc.sync.dma_start(
                        aginq_out[r0:r1, :].rearrange("(w p) f -> p w f", p=P),
                        h8ch[:, w0c : w0c + wcc, :],
                    )
                    pending_ags.setdefault(emit_w, []).append(
                        lambda: nc.gpsimd.collective_compute(
                            "AllGather", AO.bypass, replica_groups=RG,
                            ins=[aginq_out[r0:r1, :].opt()],
                            outs=[table8_out[kc][:, :].opt()],
                        ))
                elif table_out is not None:
                    nc.sync.dma_start(
                        agin_out[r0:r1, :].rearrange("(w p) f -> p w f", p=P),
                        hnew[:, w0c : w0c + wcc, :],
                    )
                    if g0 < SPLIT:
                        tgt, off = table_out[0], g0
                    else:
                        tgt, off = table_out[1], g0 - SPLIT
                    pending_ags.setdefault(emit_w, []).append(
                        lambda tgt=tgt, off=off: nc.gpsimd.collective_compute(
                            "AllGather", AO.bypass, replica_groups=RG,
                            ins=[agin_out[r0:r1, :].opt()],
                            outs=[tgt[off : off + (r1 - r0)
                                      * N_CORES, :].opt()],
                        ))

            pending_ags = {}

            def drain_ags(w):
                for fn in pending_ags.pop(w, []):
                    fn()

            # =========================================================
            # GraphConv layers
            # =========================================================
            def gc_layer(li, table_l, W_sb, agin_out, table_out, scale_ns,
                         aginq_out=None, table8_out=None, two_pass=True,
                         table8_l=None):
                """two_pass: lo-half tiles for all windows first (only needs
                the lo table chunks), then hi-half tiles + combine — lets
                this layer start before its hi table arrives. GC1's table is
                an input (no collective), so it runs single-pass for earlier
                chunk completion."""
                hnew = chunk_pool.tile([P, NW, P], BF, tag="hnew")
                h8ch = None
                if aginq_out is not None:
                    h8ch = chunk_pool.tile([P, NW, P], F8, tag="h8ch",
                                           name="h8ch")
                aggLg = chunk_pool.tile([P, NW, P + 8], BF, tag="aggLg")
                aggL = aggLg[:, :, :P]
                if two_pass:
                    halves = (("lo", (0, L)), ("hi", (L, H)))
                else:
                    halves = (("all", (0, T)),)
                for half, (t0, tn) in halves:
                    if table8_l is not None:
                        if half in ("lo", "all"):
                            upcast_half(table8_l, table_l, "lo")
                        if half in ("hi", "all"):
                            upcast_half(table8_l, table_l, "hi")
                    for (w0, nw) in spans:
                        if half == "all":
                            stg_lo = gather_span(table_l, w0, nw, False, "lo")
                            stg_hi = gather_span(table_l, w0, nw, False, "hi")
                        else:
                            stg = gather_span(table_l, w0, nw, False, half)
                        for wr in range(nw):
                            w = w0 + wr
                            if half == "all":
                                if (w - w0) % SLOAD_W == 0:
                                    sl_lo = s_en_load(
                                        w, min(SLOAD_W, nw - wr), 0, L)
                                    sl_hi = s_en_load(
                                        w, min(SLOAD_W, nw - wr), L, H)
                            else:
                                if (w - w0) % SLOAD_W == 0:
                                    sload = s_en_load(
                                        w, min(SLOAD_W, nw - wr), t0, tn)
                            swi = (w - w0) % SLOAD_W
                            aggT_full = agg_pool.tile([P, P + 8], FP,
                                                      tag="agg", name="aggT")
                            aggT = aggT_full[:, :P]
                            for k in range(tn):
                                if half == "all":
                                    if k < L:
                                        lhs = stg_lo[:, wr * L + k, :]
                                        sen = sl_lo[:, swi,
                                                    k * P : (k + 1) * P]
                                    else:
                                        lhs = stg_hi[:, wr * H + (k - L), :]
                                        sen = sl_hi[:, swi, (k - L) * P
                                                    : (k - L + 1) * P]
                                else:
                                    lhs = stg[:, wr * tn + k, :]
                                    sen = sload[:, swi, k * P : (k + 1) * P]
                                nc.tensor.matmul(
                                    out=aggT[:],
                                    lhsT=lhs,
                                    rhs=sen,
                                    start=(k == 0),
                                    stop=(k == tn - 1),
                                )
                            if half == "lo":
                                nc.scalar.copy(aggL[:, w, :], aggT[:])
                                continue
                            aggT_sb = sb_pool.tile([P, P], BF, tag="aggTsb")
                            if half == "hi":
                                nc.vector.tensor_tensor(
                                    out=aggT_sb[:], in0=aggT[:],
                                    in1=aggL[:, w, :], op=AO.add,
                                )
                            else:
                                nc.scalar.copy(aggT_sb[:], aggT[:])
                            op = mini_ps.tile([P, P], FP, tag="mini")
                            nc.tensor.matmul(out=op[:], lhsT=aggT_sb[:],
                                             rhs=W_sb[:],
                                             start=True, stop=True)
                            nc.scalar.activation(
                                hnew[:, w, :], op[:], AFT.Relu,
                                scale=ndw_sb[:, w : w + 1],
                            )
                            if scale_ns:
                                nc.vector.tensor_scalar_mul(
                                    hnew[:, w, :], hnew[:, w, :],
                                    nsw_sb[:, w : w + 1],
                                )
                            if h8ch is not None:
                                nc.scalar.copy(h8ch[:, w, :], hnew[:, w, :])
                            flush_chunk(w, hnew, agin_out, table_out,
                                        aginq_out, table8_out, h8ch)
                            drain_ags(w)
                return hnew

            # =========================================================
            # GATv2 layers
            # =========================================================
            def upcast_half(table8_l, table_l, which):
                """fp8 per-chunk tables -> bf16 gather half-table, in
                partition-major contiguous stripes (big DMA descriptors,
                cheap convert split across DVE and Act). Per-chunk source
                tensors let each upcast start as its AllGather lands."""
                nk = len(chunks)
                ks = tuple(range(nk // 2)) if which == "lo" else tuple(range(nk // 2, nk))
                tbhalf = table_l[0] if which == "lo" else table_l[1]
                base = 0
                for k in ks:
                    nrows = rc[k]
                    npiece = 2
                    rows = nrows // npiece
                    assert rows * npiece == nrows and rows % P == 0
                    for i in range(npiece):
                        per = rows  # elems per partition
                        pmax = max(rc) // 2
                        s8 = sb_pool.tile([P, pmax], F8, tag="upc8", bufs=2)
                        nc.sync.dma_start(
                            s8[:, :per],
                            table8_l[k][i * rows : (i + 1) * rows, :]
                            .rearrange("(p a) f -> p (a f)", p=P),
                        )
                        sb = sb_pool.tile([P, pmax], BF, tag="upcb", bufs=1)
                        hh = per // 2
                        nc.vector.tensor_copy(sb[:, :hh], s8[:, :hh])
                        nc.scalar.copy(sb[:, hh:per], s8[:, hh:per])
                        r0 = base + i * rows
                        nc.sync.dma_start(
                            tbhalf[r0 : r0 + rows, :].rearrange(
                                "(p a) f -> p (a f)", p=P),
                            sb[:, :per],
                        )
                    base += nrows

            def gat_layer(li, table8_l, table_l, h_prev, Ws_l, Wd_l,
                          abd_l, aginq_out, table8_out):
                # own chunk: SBUF copy of the previous layer's output tile
                # (made before this layer's hnew reuses that buffer)
                hch = chunk_pool.tile([P, NW, P], BF, tag="hch")
                nc.vector.tensor_copy(hch[:], h_prev[:])
                fdw = chunk_pool.tile([P, NW, P], F8, tag="fdw")
                for w in range(NW):
                    tp = mini_ps.tile([P, P], BF, tag="mini")
                    nc.tensor.transpose(tp[:], hch[:, w, :], ident_bf[:])
                    hwT = sb_pool.tile([P, P], BF, tag="hwTsb")
                    nc.scalar.copy(hwT[:], tp[:])
                    fp = mini_ps.tile([P, P], FP, tag="mini")
                    nc.tensor.matmul(out=fp[:], lhsT=hwT[:], rhs=Wd_l[:],
                                     start=True, stop=True)
                    nc.scalar.copy(fdw[:, w, :], fp[:])

                hnew = chunk_pool.tile([P, NW, P], BF, tag="hnew")
                h8ch = None
                if aginq_out is not None:
                    h8ch = chunk_pool.tile([P, NW, P], F8, tag="h8ch",
                                           name="h8ch")
                aggL = chunk_pool.tile([P, NW, P + 8], BF, tag="aggLg")
                TM = max(L, H)
                for half, (t0, tn) in (("lo", (0, L)), ("hi", (L, H))):
                    upcast_half(table8_l, table_l, half)
                    for (w0, nw) in spans:
                        stg = gather_span(table_l, w0, nw, True, half)
                        for wr in range(nw):
                            w = w0 + wr
                            if wr % 4 == 0:
                                n4 = min(4, nw - wr)
                                snT4 = sb_pool.tile([P, 4, TM, P], F8,
                                                    tag="snT48", bufs=2,
                                                    name="snT4")
                                nc.sync.dma_start(
                                    snT4[:, :n4, :tn, :].rearrange(
                                        "p w t f -> p w (t f)"),
                                    snden[w : w + n4, :,
                                          t0 * P : (t0 + tn) * P].rearrange(
                                        "w p f -> p w f"),
                                )
                            snTw = snT4[:, wr % 4]
                            if (w - w0) % SLOAD_W == 0:
                                sload = s_en_load(w, min(SLOAD_W, nw - wr),
                                                  t0, tn, sden8, F8,
                                                  tag="sload8")
                            swi = (w - w0) % SLOAD_W
                            agg = agg_pool.tile([P, P + 8], FP, tag="agg")

                            # phase 1 (feature-major): epsf[f', e] = Ws^T @
                            # h_cols + fdw^T @ snT in <=4-tile runs, prelu'd
                            # into elrf (still feature-major).
                            elrf = sb_pool.tile([P, TM, P], BF, tag="elr",
                                                bufs=2)
                            groups = []
                            g0 = 0
                            while g0 < tn:
                                gn = min(4, tn - g0)
                                groups.append((g0, gn))
                                g0 += gn
                            for gi, (g0, gn) in enumerate(groups):
                                epsf = ps_pool.tile([P, 4 * P], FP,
                                                    tag=f"eps{gi % 2}",
                                                    bufs=1)
                                col = (wr * tn + g0) * P
                                nc.tensor.matmul(
                                    out=epsf[:, : gn * P], lhsT=Ws_l[:],
                                    rhs=stg[:, 0, col : col + gn * P],
                                    start=True, stop=False)
                                nc.tensor.matmul(
                                    out=epsf[:, : gn * P], lhsT=fdw[:, w, :],
                                    rhs=snTw[:, g0 : g0 + gn, :].rearrange(
                                        "p a b -> p (a b)"),
                                    start=False, stop=True)
                                nc.scalar.activation(
                                    elrf[:, g0 : g0 + gn, :],
                                    epsf[:, : gn * P].rearrange(
                                        "p (a b) -> p a b", b=P),
                                    AFT.Prelu, alpha=0.2,
                                )
                            # phase 2: per-tile logits via block-diag
                            # attention matmul (edge-major out), then exp
                            logps = mini_ps.tile([P, TM * HEADS], FP,
                                                 tag="mini")
                            for k in range(tn):
                                nc.tensor.matmul(
                                    out=logps[:, k * HEADS : (k + 1) * HEADS],
                                    lhsT=elrf[:, k, :], rhs=abd_l[:],
                                    start=True, stop=True)
                            pex = sb_pool.tile([P, TM * HEADS], BF, tag="pex")
                            nc.scalar.activation(pex[:, : tn * HEADS],
                                                 logps[:, : tn * HEADS],
                                                 AFT.Exp)
                            # phase 3: fs recompute + alpha-weighting + agg
                            for gi, (g0, gn) in enumerate(groups):
                                fsps = ps_pool.tile([P, 4 * P], FP,
                                                    tag=f"fsps{gi % 2}",
                                                    bufs=1)
                                for k in range(gn):
                                    col = (wr * tn + g0 + k) * P
                                    nc.tensor.matmul(
                                        out=fsps[:, k * P : (k + 1) * P],
                                        lhsT=stg[:, 0, col : col + P],
                                        rhs=Ws_l[:], start=True, stop=True)
                                wf = sb_pool.tile([P, 4, P + 8], F8, tag="wf")
                                nc.vector.tensor_copy(
                                    wf[:, :gn, P : P + 8],
                                    pex[:, g0 * HEADS : (g0 + gn) * HEADS]
                                    .rearrange("p (a b) -> p a b", b=HEADS),
                                )
                                nc.vector.tensor_tensor(
                                    out=wf[:, :gn, 0:P].rearrange(
                                        "p a (h d) -> p a h d", d=DH
                                    ),
                                    in0=fsps[:, : gn * P].rearrange(
                                        "p (a h d) -> p a h d", h=HEADS, d=DH
                                    ),
                                    in1=pex[:, g0 * HEADS : (g0 + gn) * HEADS]
                                    .rearrange("p (a h) -> p a h", h=HEADS)
                                    .unsqueeze(3)
                                    .to_broadcast([P, gn, HEADS, DH]),
                                    op=AO.mult,
                                )
                                for k in range(gn):
                                    t = g0 + k
                                    nc.tensor.matmul(
                                        out=agg[:],
                                        lhsT=sload[:, swi,
                                                   t * P : (t + 1) * P],
                                        rhs=wf[:, k, :],
                                        start=(t == 0),
                                        stop=(t == tn - 1),
                                    )
                            if half == "lo":
                                nc.scalar.copy(aggL[:, w, :], agg[:])
                                continue
                            # ---- window flush (hi pass) ----
                            tot = sb_pool.tile([P, P + 8], FP, tag="tot", bufs=2)
                            nc.vector.tensor_tensor(
                                out=tot[:], in0=agg[:], in1=aggL[:, w, :],
                                op=AO.add,
                            )
                            sguard = sb_pool.tile([P, 8], FP, tag="sguard")
                            nc.vector.tensor_scalar_max(
                                sguard[:], tot[:, P : P + 8], 1e-30
                            )
                            rec = sb_pool.tile([P, 8], FP, tag="rec")
                            nc.vector.reciprocal(rec[:], sguard[:])
                            o1 = sb_pool.tile([P, P], FP, tag="o1")
                            nc.vector.tensor_tensor(
                                out=o1[:].rearrange("p (h d) -> p h d", d=DH),
                                in0=tot[:, 0:P].rearrange(
                                    "p (h d) -> p h d", d=DH),
                                in1=rec[:].unsqueeze(2).to_broadcast(
                                    [P, HEADS, DH]),
                                op=AO.mult,
                            )
                            nc.vector.tensor_tensor(
                                out=o1[:], in0=o1[:], in1=hch[:, w, :],
                                op=AO.add
                            )
                            nc.scalar.activation(hnew[:, w, :], o1[:],
                                                 AFT.Relu)
                            if h8ch is not None:
                                nc.scalar.copy(h8ch[:, w, :], hnew[:, w, :])
                            flush_chunk(w, hnew, None, None,
                                        aginq_out, table8_out, h8ch)
                            drain_ags(w)
                return hnew

            # =========================================================
            # forward pass
            # =========================================================
            gc_layer(0, tables[0], Wgc_sb[0], agin[0], tables[1],
                     scale_ns=True, two_pass=False)
            h2 = gc_layer(1, tables[1], Wgc_sb[1], None, None,
                          scale_ns=False, aginq_out=aginq[1],
                          table8_out=tables8[2])
            h3 = gat_layer(0, tables8[2], tables[2], h2, Ws_sb[0], Wd_sb[0],
                           abd_sb[0], aginq[2], tables8[3])
            h4 = gat_layer(1, tables8[3], tables[3], h3, Ws_sb[1], Wd_sb[1],
                           abd_sb[1], aginq[3], tables8[4])
            h5 = gat_layer(2, tables8[4], tables[4], h4, Ws_sb[2],
                           Wd_sb[2], abd_sb[2], None, None)

            # =========================================================
            # pooling + MLP (replicated)
            # =========================================================


            h5T = chunk_pool.tile([P, NW, P], BF, tag="hch")
            for w in range(NW):
                tp = mini_ps.tile([P, P], BF, tag="mini")
                nc.tensor.transpose(tp[:], h5[:, w, :], ident_bf[:])
                nc.scalar.copy(h5T[:, w, :], tp[:])

            NSEG = NW * KSEG
            stag = chunk_pool.tile([P, NSEG], FP, tag="stag")
            for w in range(NW):
                if w % 8 == 0:
                    nw8 = min(8, NW - w)
                    pmask_rep8 = sb_pool.tile(
                        [P, 8, KSEG * P], BF, tag="snT4", bufs=2,
                        name="pmask_rep8"
                    )
                    nc.sync.dma_start(
                        pmask_rep8[:, :nw8, :],
                        poolmask[w : w + nw8, :]
                        .unsqueeze(0)
                        .to_broadcast([P, nw8, KSEG * P]),
                    )
                pmask_rep = pmask_rep8[:, w % 8]
                msk = sb_pool.tile([P, KSEG, P], BF, tag="msk")
                nc.vector.tensor_tensor(
                    out=msk[:],
                    in0=h5T[:, w, :].unsqueeze(1).to_broadcast([P, KSEG, P]),
                    in1=pmask_rep[:].rearrange("p (k f) -> p k f", f=P),
                    op=AO.min,
                )
                nc.vector.tensor_reduce(
                    out=stag[:, w * KSEG : (w + 1) * KSEG], in_=msk[:],
                    axis=mybir.AxisListType.X, op=AO.max,
                )
            # graph-level masked max over segment columns -> hgT partial
            hgT_part = sb_pool.tile([P, G], FP, tag="hgT_part")
            gmask_all = sb_pool.tile([P, G, NSEG], BF, tag="dstrep4", bufs=1)
            nc.sync.dma_start(
                gmask_all[:],
                gmask[:].unsqueeze(0).to_broadcast([P, G, NSEG]),
            )
            GB = 16
            for g in range(0, G, GB):
                gm = sb_pool.tile([P, GB, NSEG], FP, tag="gm", bufs=2)
                nc.vector.tensor_tensor(
                    out=gm[:],
                    in0=stag[:, :NSEG].unsqueeze(1).to_broadcast([P, GB, NSEG]),
                    in1=gmask_all[:, g : g + GB], op=AO.min,
                )
                nc.vector.tensor_reduce(
                    out=hgT_part[:, g : g + GB], in_=gm[:],
                    axis=mybir.AxisListType.X, op=AO.max,
                )
            nc.sync.dma_start(hgpart[:], hgT_part[:])
            nc.gpsimd.collective_compute(
                "AllGather", AO.bypass, replica_groups=RG,
                ins=[hgpart.ap().opt()], outs=[hgall.ap().opt()],
            )
            # final max over ranks: hgall rows = (r p)
            hgl = sb_pool.tile([P, N_CORES * G], FP, tag="hgl")
            nc.sync.dma_start(
                hgl[:].rearrange("p (r g) -> p r g", g=G),
                hgall[:].rearrange("(r p) g -> p r g", p=P),
            )
            hgT = sb_pool.tile([P, G], FP, tag="hgT")
            nc.vector.tensor_reduce(
                out=hgT[:],
                in_=hgl[:].rearrange("p (r g) -> p g r", g=G),
                axis=mybir.AxisListType.X, op=AO.max,
            )

            Wc1_sb = load_const(Wc1, [P, P], FP)
            Wc2_sb = load_const(Wc2, [P, 64], FP)
            Wc3_sb = load_const(Wc3, [64, OUT], FP)

            z1p = mini_ps.tile([G, P], FP, tag="mini")
            nc.tensor.matmul(out=z1p[:], lhsT=hgT[:], rhs=Wc1_sb[:],
                             start=True, stop=True)
            z1 = sb_pool.tile([G, P], FP, tag="z1")
            nc.scalar.activation(z1[:], z1p[:], AFT.Relu)
            z1Tp = mini_ps.tile([P, G], FP, tag="mini")
            nc.tensor.transpose(z1Tp[:], z1[:], ident_f[:G, :G])
            z1T = sb_pool.tile([P, G], FP, tag="z1T")
            nc.scalar.copy(z1T[:], z1Tp[:])
            z2p = mini_ps.tile([G, 64], FP, tag="mini")
            nc.tensor.matmul(out=z2p[:], lhsT=z1T[:], rhs=Wc2_sb[:],
                             start=True, stop=True)
            z2 = sb_pool.tile([G, 64], FP, tag="z2")
            nc.scalar.activation(z2[:], z2p[:], AFT.Relu)
            z2Tp = mini_ps.tile([64, G], FP, tag="mini")
            nc.tensor.transpose(z2Tp[:], z2[:], ident_f[:G, :G])
            z2T = sb_pool.tile([64, G], FP, tag="z2T")
            nc.scalar.copy(z2T[:], z2Tp[:])
            z3p = mini_ps.tile([G, OUT], FP, tag="mini")
            nc.tensor.matmul(out=z3p[:], lhsT=z2T[:], rhs=Wc3_sb[:],
                             start=True, stop=True)
            z3 = sb_pool.tile([G, OUT], FP, tag="z3")
            nc.scalar.copy(z3[:], z3p[:])
            nc.sync.dma_start(out_ext[:], z3[:])

    nc.compile()
    return nc


# ---------------------------------------------------------------------------
# Entry point
# ---------------------------------------------------------------------------

def _run(inputs, nw_per_core=49, trace=False):
    from concourse.bass_utils import run_bass_kernel_spmd

    src = np.asarray(inputs["src"])
    dst = np.asarray(inputs["dst"])
    n2g = np.asarray(inputs["node2graph"])
    feat = np.asarray(inputs["feature"], np.float32)

    cfg, per_core, ns, nd, perm_src = prep(src, dst, n2g, nw_per_core)
    NP = cfg["NP"]

    featp = np.zeros((NP, P), np.float32)
    featp[: feat.shape[0]] = feat
    featp *= ns[:, None]
    table0 = np.ascontiguousarray(featp[perm_src]).astype(bf16)

    def b(x):
        return np.ascontiguousarray(np.asarray(x, np.float32).astype(bf16))

    SPLIT = cfg["SPLIT"]
    common = dict(
        table0lo=np.ascontiguousarray(table0[:SPLIT]),
        table0hi=np.ascontiguousarray(table0[SPLIT:]),
        Wgc0=b(inputs["W_gc1"]), Wgc1=b(inputs["W_gc2"]),
        Wc1=np.ascontiguousarray(np.asarray(inputs["Wc1"], np.float32)),
        Wc2=np.ascontiguousarray(np.asarray(inputs["Wc2"], np.float32)),
        Wc3=np.ascontiguousarray(np.asarray(inputs["Wc3"], np.float32)),
    )
    attn = np.asarray(inputs["attn"], np.float32)
    for i in range(3):
        common[f"Ws{i}"] = b(np.asarray(inputs["W_src"], np.float32)[i])
        common[f"Wd{i}"] = b(np.asarray(inputs["W_dst"], np.float32)[i])
        ar = np.zeros((P, HEADS), np.float32)
        for h in range(HEADS):
            ar[h * DH : (h + 1) * DH, h] = attn[i][h]
        common[f"abd{i}"] = np.ascontiguousarray(ar).astype(bf16)

    in_maps = []
    for c in range(N_CORES):
        m = dict(common)
        m.update(per_core[c])
        in_maps.append(m)

    nc = build_nc(cfg)
    res = run_bass_kernel_spmd(nc, in_maps, core_ids=list(range(N_CORES)),
                               trace=trace)
    return np.asarray(res.results[0]["out"], np.float32), res


def kernel(**inputs) -> np.ndarray:
    out, _ = _run(inputs)
    return out



# revision 52
# speedup vs baseline: 1.0056x; 1.0056x over previous
"""Trainium2 Bass kernel for nn_DifferentPooling (GNN message passing).

Strategy (8 NeuronCores, SPMD):
  - Nodes padded to NP = 8*CHUNK, partitioned by node id across cores in a
    K=4 chunk-permuted address space; edges partitioned by dst core and
    bucketed into 128-node dst windows. Aggregations (segment sum /
    softmax-sum) run on the TensorEngine as one-hot matmuls; the one-hot
    S_en / S_en^T matrices are built host-side and streamed from DRAM.
  - Per layer the 8 per-core feature chunks are AllGather'd (4 chunks,
    issued as producing windows finish). The GAT-feeding tables travel as
    fp8-e4m3 (half collective bytes on the serial collective device) and
    are upcast to bf16 gather tables on the consumer in partition-major
    stripes (DVE+Act split). The GC2 table is precision-critical and stays
    bf16.
  - Layers (except GC1, whose table is the kernel input) process all
    windows in two passes: lo-half edge tiles first (only needs the lo
    half of the incoming table, i.e. the first 2 AllGather chunks), then
    hi-half tiles + combine with the saved lo partials - overlapping each
    layer's start with its predecessor's trailing collectives.
  - GATv2: eps/leaky-relu computed feature-major (weight-stationary
    matmuls, N=512), logits via a block-diagonal attention matmul
    (edge-major out), exp on Act; alpha-weighting and the [num | den]
    aggregation stay edge-major with den in 8 extra PSUM columns.
  - Softmax uses exp(logit) without max subtraction (logits tiny) with a
    1e-30 guard; graph max-pooling via masked-max segments and a small
    AllGather; replicated fp32 MLP head.

All biases in this problem are zeros by spec (fill="zeros"); they are not
applied on device.
"""

import sys

sys.path.insert(0, "/opt/trn_rl_repo")

import numpy as np
import ml_dtypes

bf16 = ml_dtypes.bfloat16
fp8 = ml_dtypes.float8_e4m3fn

N_CORES = 8
P = 128  # window size / partition count
N_REAL = 50000
E_REAL = 500000
G = 64
HID = 128
HEADS = 8
DH = 16
OUT = 256


# ---------------------------------------------------------------------------
# Host-side preprocessing
# ---------------------------------------------------------------------------

def _wrap_idx(arr):
    """int idx array (len % 16 == 0) -> [128, len/16] int16 wrapped layout:
    idx i lives at [i % 16, i // 16], replicated across the 8 groups of 16
    partitions (one per Q7 core)."""
    a = np.asarray(arr, np.int16).reshape(-1, 16).T  # [16, cols]
    return np.tile(a, (8, 1))  # [128, cols]


def prep(src, dst, node2graph, nw_per_core, kchunks=4):
    """Build per-core edge/window metadata. Returns (cfg, host arrays).

    The node table on device lives in a chunk-permuted layout so each
    layer's AllGather can be split into `kchunks` window-chunks issued as
    soon as the producing windows finish: address order is
    [chunk][rank][window-in-chunk][pos], matching what chunked AllGathers
    over agin row-slices naturally produce. All gather indices below are
    built in permuted address space.
    """
    NW = nw_per_core
    CHUNK = NW * P
    NP = N_CORES * CHUNK
    HALF = NP // 2
    N = len(node2graph)
    E = len(src)

    src = np.asarray(src, np.int64)
    dst = np.asarray(dst, np.int64)
    n2g = np.asarray(node2graph, np.int64)

    # chunk structure (in windows): even split measured best (smaller-first
    # and K=5 both regressed against the cost model's 15us/chunk overhead).
    K = kchunks
    wc = [NW // K] * K
    for c in range(NW - sum(wc)):
        wc[c] += 1
    w0s = np.cumsum([0] + wc)[:-1]
    chunk_of_win = np.repeat(np.arange(K), wc)
    cbase = N_CORES * P * np.cumsum([0] + wc)[:-1]
    # lo/hi gather split at the chunk K/2 boundary so the lo and hi table
    # halves can live in separate DRAM tensors (precise collective deps)
    SPLIT = int(cbase[K // 2]) if K >= 2 else HALF

    def paddr(n):
        r, local = np.divmod(np.asarray(n, np.int64), CHUNK)
        w, p = np.divmod(local, P)
        c = chunk_of_win[w]
        return cbase[c] + (r * np.asarray(wc)[c] + (w - w0s[c])) * P + p

    outdeg = np.zeros(NP, np.float32)
    np.add.at(outdeg, src, 1.0)
    indeg = np.zeros(NP, np.float32)
    np.add.at(indeg, dst, 1.0)
    ns = np.maximum(outdeg, 1.0) ** -0.5
    nd = np.maximum(indeg, 1.0) ** -0.5

    # sort edges by dst, bucket into windows; src ids move to permuted space
    order = np.argsort(dst, kind="stable")
    sdst = dst[order]
    ssrc = paddr(src[order])
    n_win_total = NP // P
    win_starts = np.searchsorted(sdst, np.arange(0, NP + 1, P))

    # per (global window): lo/hi edge lists sorted by src
    lo_lists, hi_lists = [], []
    max_lo = max_hi = 1
    for w in range(n_win_total):
        a, b = win_starts[w], win_starts[w + 1]
        es, ed = ssrc[a:b], sdst[a:b] - w * P
        m = es < SPLIT
        ordl = np.argsort(es[m], kind="stable")
        ordh = np.argsort(es[~m], kind="stable")
        lo_lists.append((es[m][ordl], ed[m][ordl]))
        hi_lists.append((es[~m][ordh] - SPLIT, ed[~m][ordh]))
        max_lo = max(max_lo, len(lo_lists[-1][0]))
        max_hi = max(max_hi, len(hi_lists[-1][0]))

    L = (max_lo + P - 1) // P
    H = (max_hi + P - 1) // P
    T = L + H

    # spans of SPAN_W windows (gather batching granularity)
    SPAN_W = 8 if NW >= 8 else 2
    spans = []
    w0 = 0
    while w0 < NW:
        spans.append((w0, min(SPAN_W, NW - w0)))
        w0 += SPAN_W

    per_core = []
    for c in range(N_CORES):
        idx_lo = np.zeros((NW, L * P), np.int64)
        dst_lo = np.full((NW, L * P), P, np.int64)  # sentinel 128
        idx_hi = np.zeros((NW, H * P), np.int64)
        dst_hi = np.full((NW, H * P), P, np.int64)
        for w in range(NW):
            el, dl = lo_lists[c * NW + w]
            eh, dh_ = hi_lists[c * NW + w]
            idx_lo[w, : len(el)] = el
            dst_lo[w, : len(dl)] = dl
            idx_hi[w, : len(eh)] = eh
            dst_hi[w, : len(dh_)] = dh_
        # dstloc: [NW*T, 128] -> transpose to [128, NW*T]; col w*T+t
        dstloc = np.concatenate(
            [dst_lo.reshape(NW, L, P), dst_hi.reshape(NW, H, P)], axis=1
        ).reshape(NW * T, P)
        ndw = nd[c * CHUNK : (c + 1) * CHUNK].reshape(NW, P).T.copy()
        nsw = ns[c * CHUNK : (c + 1) * CHUNK].reshape(NW, P).T.copy()
        # host-built one-hot selection matrices:
        # sden[w, e, t*128+n] = (dst-pos of edge slot (w,t,e) == n)
        # snden[w, p, t*128+e] = sden[w, e, t*128+p]  (transpose)
        eye = np.zeros((P + 1, P), bf16)
        eye[np.arange(P), np.arange(P)] = 1
        oh = eye[dstloc]                       # [NW*T, P(e), P(n)]
        sden_h = np.ascontiguousarray(
            oh.reshape(NW, T, P, P).transpose(0, 2, 1, 3)
            .reshape(NW, P, T * P))
        snden_h = np.ascontiguousarray(
            oh.reshape(NW, T, P, P).transpose(0, 3, 1, 2)
            .reshape(NW, P, T * P))
        per_core.append(
            dict(
                idx_lo=_wrap_idx(idx_lo.reshape(-1)),
                idx_hi=_wrap_idx(idx_hi.reshape(-1)),
                sden=sden_h,
                sden8=sden_h.astype(fp8),
                snden=snden_h.astype(fp8),
                ndw=np.ascontiguousarray(ndw, np.float32),
                nsw=np.ascontiguousarray(nsw, np.float32),
            )
        )

    # pooling segments per core: runs of equal graph id inside each window
    n2g_pad = np.full(NP, -1, np.int64)
    n2g_pad[:N] = n2g
    seg_all = []  # per core: list of (w, j0, j1, g)
    KSEG = 1
    for c in range(N_CORES):
        segs = []
        for w in range(NW):
            ids = n2g_pad[c * CHUNK + w * P : c * CHUNK + (w + 1) * P]
            j = 0
            wsegs = []
            while j < P:
                g = ids[j]
                k = j
                while k < P and ids[k] == g:
                    k += 1
                if g >= 0:
                    wsegs.append((j, k, int(g)))
                j = k
            KSEG = max(KSEG, len(wsegs))
            segs.append(wsegs)
        seg_all.append(segs)

    BIG = np.float32(1e30)
    NSEG = NW * KSEG
    for c in range(N_CORES):
        maskvec = np.full((NW, KSEG, P), -BIG, np.float32)
        gmask = np.full((G, NSEG), -BIG, np.float32)
        for w in range(NW):
            for k, (j0, j1, g) in enumerate(seg_all[c][w]):
                maskvec[w, k, j0:j1] = BIG
                gmask[g, w * KSEG + k] = BIG
        per_core[c]["poolmask"] = maskvec.reshape(NW, KSEG * P).astype(bf16)
        per_core[c]["gmask"] = gmask.astype(bf16)

    cfg = dict(NW=NW, CHUNK=CHUNK, NP=NP, HALF=HALF, SPLIT=SPLIT,
               L=L, H=H, T=T, spans=spans, KSEG=KSEG,
               chunks=list(zip(w0s.tolist(), wc)))
    perm_src = np.empty(NP, np.int64)  # perm_src[paddr] = original id
    perm_src[paddr(np.arange(NP))] = np.arange(NP)
    return cfg, per_core, ns, nd, perm_src


# ---------------------------------------------------------------------------
# Bass kernel builder
# ---------------------------------------------------------------------------

def build_nc(cfg):
    import concourse.bacc as bacc
    import concourse.bass as bass
    import concourse.mybir as mybir
    import concourse.tile as tile
    from concourse.masks import make_identity

    NW, CHUNK, NP, HALF = cfg["NW"], cfg["CHUNK"], cfg["NP"], cfg["HALF"]
    L, H, T, spans, KSEG = cfg["L"], cfg["H"], cfg["T"], cfg["spans"], cfg["KSEG"]
    chunks = cfg["chunks"]
    SPLIT = cfg["SPLIT"]
    FP = mybir.dt.float32
    BF = mybir.dt.bfloat16
    F8 = mybir.dt.float8e4
    AO = mybir.AluOpType
    AFT = mybir.ActivationFunctionType

    nc = bacc.Bacc("TRN2", target_bir_lowering=False, debug=False,
                   num_devices=N_CORES)

    def din(name, shape, dt=BF):
        return nc.dram_tensor(name, shape, dt, kind="ExternalInput")

    table0 = (din("table0lo", [SPLIT, P]), din("table0hi", [NP - SPLIT, P]))
    Wgc = [din(f"Wgc{i}", [P, P]) for i in range(2)]
    Ws = [din(f"Ws{i}", [P, P]) for i in range(3)]
    Wd = [din(f"Wd{i}", [P, P]) for i in range(3)]
    abd = [din(f"abd{i}", [P, HEADS]) for i in range(3)]
    Wc1 = din("Wc1", [P, P], FP)
    Wc2 = din("Wc2", [P, 64], FP)
    Wc3 = din("Wc3", [64, OUT], FP)
    idx_lo = din("idx_lo", [P, NW * L * P // 16], mybir.dt.int16)
    idx_hi = din("idx_hi", [P, NW * H * P // 16], mybir.dt.int16)
    sden = din("sden", [NW, P, T * P])
    sden8 = din("sden8", [NW, P, T * P], F8)
    snden = din("snden", [NW, P, T * P], F8)
    ndw = din("ndw", [P, NW], FP)
    nsw = din("nsw", [P, NW], FP)
    poolmask = din("poolmask", [NW, KSEG * P])
    gmask = din("gmask", [G, NW * KSEG])

    out_ext = nc.dram_tensor("out", [G, OUT], FP, kind="ExternalOutput")

    # internal DRAM
    # agin: bf16 own-chunk copies (hch source); aginq: fp8 collective inputs
    # for the GAT-feeding layers (tables 2..4 travel as fp8 and are upcast
    # to bf16 gather tables on the consumer side).
    agin = [nc.dram_tensor("agin0", [CHUNK, P], BF)]
    aginq = [nc.dram_tensor(f"aginq{i}", [CHUNK, P], F8) for i in range(4)]
    tables = [table0,
              (nc.dram_tensor("tlo1", [SPLIT, P], BF, addr_space="Shared"),
               nc.dram_tensor("thi1", [NP - SPLIT, P], BF,
                              addr_space="Shared"))]
    rc = [wcc * P * N_CORES for (_w0c, wcc) in chunks]
    tables8 = [None, None]
    for i in range(2, 5):
        tables8.append(tuple(
            nc.dram_tensor(f"t8c{i}_{k}", [rc[k], P], F8,
                           addr_space="Shared")
            for k in range(len(chunks))))
        tables.append(
            (nc.dram_tensor(f"tlo{i}", [SPLIT, P], BF),
             nc.dram_tensor(f"thi{i}", [NP - SPLIT, P], BF)))
    hgpart = nc.dram_tensor("hgpart", [P, G], FP)
    hgall = nc.dram_tensor("hgall", [N_CORES * P, G], FP, addr_space="Shared")

    RG = [list(range(N_CORES))]

    with tile.TileContext(nc) as tc:
        import contextlib

        ctx = contextlib.ExitStack()
        with ctx:
            const_pool = ctx.enter_context(tc.tile_pool(name="const", bufs=1))
            stg_pool = ctx.enter_context(tc.tile_pool(name="stg", bufs=2))
            sb_pool = ctx.enter_context(tc.tile_pool(name="sb", bufs=3))
            chunk_pool = ctx.enter_context(tc.tile_pool(name="chunk", bufs=1))
            ps_pool = ctx.enter_context(
                tc.tile_pool(name="ps", bufs=2, space="PSUM")
            )
            snt_pool = ctx.enter_context(
                tc.tile_pool(name="snt", bufs=1, space="PSUM")
            )
            agg_pool = ctx.enter_context(
                tc.tile_pool(name="agg", bufs=2, space="PSUM")
            )
            mini_ps = ctx.enter_context(
                tc.tile_pool(name="minips", bufs=2, space="PSUM")
            )

            # --- constants in SBUF ---
            ident_bf = const_pool.tile([P, P], BF, tag="identbf")
            make_identity(nc, ident_bf[:])
            ident_f = const_pool.tile([P, P], FP, tag="identf")
            make_identity(nc, ident_f[:])

            def load_const(h, shape, dt=BF, tag=None):
                t = const_pool.tile(shape, dt, tag=tag or h.name)
                nc.sync.dma_start(t[:], h[:])
                return t

            Wgc_sb = [load_const(w, [P, P]) for w in Wgc]
            Ws_sb = [load_const(w, [P, P]) for w in Ws]
            Wd_sb = [load_const(w, [P, P]) for w in Wd]
            abd_sb = [load_const(w, [P, HEADS]) for w in abd]
            ndw_sb = load_const(ndw, [P, NW], FP)
            nsw_sb = load_const(nsw, [P, NW], FP)
            idxlo_sb = load_const(idx_lo, [P, NW * L * P // 16], mybir.dt.int16)
            idxhi_sb = load_const(idx_hi, [P, NW * H * P // 16], mybir.dt.int16)

            SLOAD_W = 4  # windows per S_en reload DMA

            def s_en_load(w0, nwin, t0, tn, src_t=None, dt=None, tag="sload"):
                """Load S_en tiles [t0, t0+tn) for windows [w0, w0+nwin)."""
                if src_t is None:
                    src_t, dt = sden, BF
                sload = sb_pool.tile([P, SLOAD_W, max(L, H) * P], dt,
                                     tag=tag, bufs=2)
                nc.sync.dma_start(
                    sload[:, :nwin, : tn * P],
                    src_t[w0 : w0 + nwin, :,
                          t0 * P : (t0 + tn) * P].rearrange("w p f -> p w f"),
                )
                return sload

            def gather_span(table_l, w0, nw, transpose, which):
                """Gather the lo or hi edges of windows [w0, w0+nw).
                transpose -> [128, 1, n] column tiles, else
                [128, ntiles, 128] row tiles."""
                if which == "lo":
                    n, idx_sb, colpos = nw * L * P, idxlo_sb, w0 * L * P
                    half = table_l[0][0:SPLIT, :]
                else:
                    n, idx_sb, colpos = nw * H * P, idxhi_sb, w0 * H * P
                    half = table_l[1][0 : NP - SPLIT, :]
                nmax = nw * max(L, H) * P
                if transpose:
                    t = stg_pool.tile([P, 1, nmax], BF, tag="stg", bufs=3)
                    t = t[:, :, :n]
                else:
                    t = stg_pool.tile([P, nmax // P, P], BF, tag="stg", bufs=3)
                    t = t[:, : n // P, :]
                nc.gpsimd.dma_gather(
                    t[:, :, :],
                    half,
                    idx_sb[:, colpos // 16 : (colpos + n) // 16],
                    n,
                    n,
                    P,
                    transpose=transpose,
                    single_packet=False,
                )
                return t

            chunk_end = {w0c + wcc - 1: (w0c, wcc) for (w0c, wcc) in chunks}

            def flush_chunk(w, hnew, agin_out, table_out, aginq_out=None,
                            table8_out=None):
                """After window w completes, DMA the finished chunk's rows
                out (bf16 agin copy and/or fp8 aginq). The AllGather itself
                is emitted a few windows LATER (see drain_ags): collectives
                issue from the Pool sequencer, and a collective emitted
                right at chunk completion holds Pool.SEQ waiting for the
                chunk DMA - blocking the remaining span gathers behind it.
                Delaying emission lets it issue with its input already in
                DRAM (short SEQ hold) while still starting early."""
                if w not in chunk_end:
                    return
                if aginq_out is None and agin_out is None:
                    return
                w0c, wcc = chunk_end[w]
                r0, r1 = w0c * P, (w0c + wcc) * P
                g0 = r0 * N_CORES
                kc = [i for i, (a, _b) in enumerate(chunks) if a == w0c][0]
                emit_w = min(w + 3, NW - 1)
                if table8_out is not None:
                    h8 = sb_pool.tile([P, (NW + len(chunks) - 1)
                                       // len(chunks), P], F8, tag="h8",
                                      bufs=2)
                    nc.vector.tensor_copy(
                        h8[:, :wcc, :], hnew[:, w0c : w0c + wcc, :])
                    nc.sync.dma_start(
                        aginq_out[r0:r1, :].rearrange("(w p) f -> p w f", p=P),
                        h8[:, :wcc, :],
                    )
                    pending_ags.setdefault(emit_w, []).append(
                        lambda: nc.gpsimd.collective_compute(
                            "AllGather", AO.bypass, replica_groups=RG,
                            ins=[aginq_out[r0:r1, :].opt()],
                            outs=[table8_out[kc][:, :].opt()],
                        ))
                elif table_out is not None:
                    nc.sync.dma_start(
                        agin_out[r0:r1, :].rearrange("(w p) f -> p w f", p=P),
                        hnew[:, w0c : w0c + wcc, :],
                    )
                    if g0 < SPLIT:
                        tgt, off = table_out[0], g0
                    else:
                        tgt, off = table_out[1], g0 - SPLIT
                    pending_ags.setdefault(emit_w, []).append(
                        lambda tgt=tgt, off=off: nc.gpsimd.collective_compute(
                            "AllGather", AO.bypass, replica_groups=RG,
                            ins=[agin_out[r0:r1, :].opt()],
                            outs=[tgt[off : off + (r1 - r0)
                                      * N_CORES, :].opt()],
                        ))

            pending_ags = {}

            def drain_ags(w):
                for fn in pending_ags.pop(w, []):
                    fn()

            # =========================================================
            # GraphConv layers
            # =========================================================
            def gc_layer(li, table_l, W_sb, agin_out, table_out, scale_ns,
                         aginq_out=None, table8_out=None, two_pass=True,
                         table8_l=None):
                """two_pass: lo-half tiles for all windows first (only needs
                the lo table chunks), then hi-half tiles + combine — lets
                this layer start before its hi table arrives. GC1's table is
                an input (no collective), so it runs single-pass for earlier
                chunk completion."""
                hnew = chunk_pool.tile([P, NW, P], BF, tag="hnew")
                aggLg = chunk_pool.tile([P, NW, P + 8], BF, tag="aggLg")
                aggL = aggLg[:, :, :P]
                if two_pass:
                    halves = (("lo", (0, L)), ("hi", (L, H)))
                else:
                    halves = (("all", (0, T)),)
                for half, (t0, tn) in halves:
                    if table8_l is not None:
                        if half in ("lo", "all"):
                            upcast_half(table8_l, table_l, "lo")
                        if half in ("hi", "all"):
                            upcast_half(table8_l, table_l, "hi")
                    for (w0, nw) in spans:
                        if half == "all":
                            stg_lo = gather_span(table_l, w0, nw, False, "lo")
                            stg_hi = gather_span(table_l, w0, nw, False, "hi")
                        else:
                            stg = gather_span(table_l, w0, nw, False, half)
                        for wr in range(nw):
                            w = w0 + wr
                            if half == "all":
                                if (w - w0) % SLOAD_W == 0:
                                    sl_lo = s_en_load(
                                        w, min(SLOAD_W, nw - wr), 0, L)
                                    sl_hi = s_en_load(
                                        w, min(SLOAD_W, nw - wr), L, H)
                            else:
                                if (w - w0) % SLOAD_W == 0:
                                    sload = s_en_load(
                                        w, min(SLOAD_W, nw - wr), t0, tn)
                            swi = (w - w0) % SLOAD_W
                            aggT_full = agg_pool.tile([P, P + 8], FP,
                                                      tag="agg", name="aggT")
                            aggT = aggT_full[:, :P]
                            for k in range(tn):
                                if half == "all":
                                    if k < L:
                                        lhs = stg_lo[:, wr * L + k, :]
                                        sen = sl_lo[:, swi,
                                                    k * P : (k + 1) * P]
                                    else:
                                        lhs = stg_hi[:, wr * H + (k - L), :]
                                        sen = sl_hi[:, swi, (k - L) * P
                                                    : (k - L + 1) * P]
                                else:
                                    lhs = stg[:, wr * tn + k, :]
                                    sen = sload[:, swi, k * P : (k + 1) * P]
                                nc.tensor.matmul(
                                    out=aggT[:],
                                    lhsT=lhs,
                                    rhs=sen,
                                    start=(k == 0),
                                    stop=(k == tn - 1),
                                )
                            if half == "lo":
                                nc.scalar.copy(aggL[:, w, :], aggT[:])
                                continue
                            aggT_sb = sb_pool.tile([P, P], BF, tag="aggTsb")
                            if half == "hi":
                                nc.vector.tensor_tensor(
                                    out=aggT_sb[:], in0=aggT[:],
                                    in1=aggL[:, w, :], op=AO.add,
                                )
                            else:
                                nc.scalar.copy(aggT_sb[:], aggT[:])
                            op = mini_ps.tile([P, P], FP, tag="mini")
                            nc.tensor.matmul(out=op[:], lhsT=aggT_sb[:],
                                             rhs=W_sb[:],
                                             start=True, stop=True)
                            nc.scalar.activation(
                                hnew[:, w, :], op[:], AFT.Relu,
                                scale=ndw_sb[:, w : w + 1],
                            )
                            if scale_ns:
                                nc.vector.tensor_scalar_mul(
                                    hnew[:, w, :], hnew[:, w, :],
                                    nsw_sb[:, w : w + 1],
                                )
                            flush_chunk(w, hnew, agin_out, table_out,
                                        aginq_out, table8_out)
                            drain_ags(w)
                return hnew

            # =========================================================
            # GATv2 layers
            # =========================================================
            def upcast_half(table8_l, table_l, which):
                """fp8 per-chunk tables -> bf16 gather half-table, in
                partition-major contiguous stripes (big DMA descriptors,
                cheap convert split across DVE and Act). Per-chunk source
                tensors let each upcast start as its AllGather lands."""
                nk = len(chunks)
                ks = tuple(range(nk // 2)) if which == "lo" else tuple(range(nk // 2, nk))
                tbhalf = table_l[0] if which == "lo" else table_l[1]
                base = 0
                for k in ks:
                    nrows = rc[k]
                    npiece = 2
                    rows = nrows // npiece
                    assert rows * npiece == nrows and rows % P == 0
                    for i in range(npiece):
                        per = rows  # elems per partition
                        pmax = max(rc) // 2
                        s8 = sb_pool.tile([P, pmax], F8, tag="upc8", bufs=2)
                        nc.sync.dma_start(
                            s8[:, :per],
                            table8_l[k][i * rows : (i + 1) * rows, :]
                            .rearrange("(p a) f -> p (a f)", p=P),
                        )
                        sb = sb_pool.tile([P, pmax], BF, tag="upcb", bufs=1)
                        hh = per // 2
                        nc.vector.tensor_copy(sb[:, :hh], s8[:, :hh])
                        nc.scalar.copy(sb[:, hh:per], s8[:, hh:per])
                        r0 = base + i * rows
                        nc.sync.dma_start(
                            tbhalf[r0 : r0 + rows, :].rearrange(
                                "(p a) f -> p (a f)", p=P),
                            sb[:, :per],
                        )
                    base += nrows

            def gat_layer(li, table8_l, table_l, h_prev, Ws_l, Wd_l,
                          abd_l, aginq_out, table8_out):
                # own chunk: SBUF copy of the previous layer's output tile
                # (made before this layer's hnew reuses that buffer)
                hch = chunk_pool.tile([P, NW, P], BF, tag="hch")
                nc.vector.tensor_copy(hch[:], h_prev[:])
                fdw = chunk_pool.tile([P, NW, P], F8, tag="fdw")
                for w in range(NW):
                    tp = mini_ps.tile([P, P], BF, tag="mini")
                    nc.tensor.transpose(tp[:], hch[:, w, :], ident_bf[:])
                    hwT = sb_pool.tile([P, P], BF, tag="hwTsb")
                    nc.scalar.copy(hwT[:], tp[:])
                    fp = mini_ps.tile([P, P], FP, tag="mini")
                    nc.tensor.matmul(out=fp[:], lhsT=hwT[:], rhs=Wd_l[:],
                                     start=True, stop=True)
                    nc.scalar.copy(fdw[:, w, :], fp[:])

                hnew = chunk_pool.tile([P, NW, P], BF, tag="hnew")
                aggL = chunk_pool.tile([P, NW, P + 8], BF, tag="aggLg")
                TM = max(L, H)
                for half, (t0, tn) in (("lo", (0, L)), ("hi", (L, H))):
                    upcast_half(table8_l, table_l, half)
                    for (w0, nw) in spans:
                        stg = gather_span(table_l, w0, nw, True, half)
                        for wr in range(nw):
                            w = w0 + wr
                            if wr % 4 == 0:
                                n4 = min(4, nw - wr)
                                snT4 = sb_pool.tile([P, 4, TM, P], F8,
                                                    tag="snT48", bufs=2,
                                                    name="snT4")
                                nc.sync.dma_start(
                                    snT4[:, :n4, :tn, :].rearrange(
                                        "p w t f -> p w (t f)"),
                                    snden[w : w + n4, :,
                                          t0 * P : (t0 + tn) * P].rearrange(
                                        "w p f -> p w f"),
                                )
                            snTw = snT4[:, wr % 4]
                            if (w - w0) % SLOAD_W == 0:
                                sload = s_en_load(w, min(SLOAD_W, nw - wr),
                                                  t0, tn, sden8, F8,
                                                  tag="sload8")
                            swi = (w - w0) % SLOAD_W
                            agg = agg_pool.tile([P, P + 8], FP, tag="agg")

                            # phase 1 (feature-major): epsf[f', e] = Ws^T @
                            # h_cols + fdw^T @ snT in <=4-tile runs, prelu'd
                            # into elrf (still feature-major).
                            elrf = sb_pool.tile([P, TM, P], BF, tag="elr",
                                                bufs=2)
                            groups = []
                            g0 = 0
                            while g0 < tn:
                                gn = min(4, tn - g0)
                                groups.append((g0, gn))
                                g0 += gn
                            for gi, (g0, gn) in enumerate(groups):
                                epsf = ps_pool.tile([P, 4 * P], FP,
                                                    tag=f"eps{gi % 2}",
                                                    bufs=1)
                                col = (wr * tn + g0) * P
                                nc.tensor.matmul(
                                    out=epsf[:, : gn * P], lhsT=Ws_l[:],
                                    rhs=stg[:, 0, col : col + gn * P],
                                    start=True, stop=False)
                                nc.tensor.matmul(
                                    out=epsf[:, : gn * P], lhsT=fdw[:, w, :],
                                    rhs=snTw[:, g0 : g0 + gn, :].rearrange(
                                        "p a b -> p (a b)"),
                                    start=False, stop=True)
                                nc.scalar.activation(
                                    elrf[:, g0 : g0 + gn, :],
                                    epsf[:, : gn * P].rearrange(
                                        "p (a b) -> p a b", b=P),
                                    AFT.Prelu, alpha=0.2,
                                )
                            # phase 2: per-tile logits via block-diag
                            # attention matmul (edge-major out), then exp
                            logps = mini_ps.tile([P, TM * HEADS], FP,
                                                 tag="mini")
                            for k in range(tn):
                                nc.tensor.matmul(
                                    out=logps[:, k * HEADS : (k + 1) * HEADS],
                                    lhsT=elrf[:, k, :], rhs=abd_l[:],
                                    start=True, stop=True)
                            pex = sb_pool.tile([P, TM * HEADS], BF, tag="pex")
                            nc.scalar.activation(pex[:, : tn * HEADS],
                                                 logps[:, : tn * HEADS],
                                                 AFT.Exp)
                            # phase 3: fs recompute + alpha-weighting + agg
                            for gi, (g0, gn) in enumerate(groups):
                                fsps = ps_pool.tile([P, 4 * P], FP,
                                                    tag=f"fsps{gi % 2}",
                                                    bufs=1)
                                for k in range(gn):
                                    col = (wr * tn + g0 + k) * P
                                    nc.tensor.matmul(
                                        out=fsps[:, k * P : (k + 1) * P],
                                        lhsT=stg[:, 0, col : col + P],
                                        rhs=Ws_l[:], start=True, stop=True)
                                wf = sb_pool.tile([P, 4, P + 8], F8, tag="wf")
                                nc.vector.tensor_copy(
                                    wf[:, :gn, P : P + 8],
                                    pex[:, g0 * HEADS : (g0 + gn) * HEADS]
                                    .rearrange("p (a b) -> p a b", b=HEADS),
                                )
                                nc.vector.tensor_tensor(
                                    out=wf[:, :gn, 0:P].rearrange(
                                        "p a (h d) -> p a h d", d=DH
                                    ),
                                    in0=fsps[:, : gn * P].rearrange(
                                        "p (a h d) -> p a h d", h=HEADS, d=DH
                                    ),
                                    in1=pex[:, g0 * HEADS : (g0 + gn) * HEADS]
                                    .rearrange("p (a h) -> p a h", h=HEADS)
                                    .unsqueeze(3)
                                    .to_broadcast([P, gn, HEADS, DH]),
                                    op=AO.mult,
                                )
                                for k in range(gn):
                                    t = g0 + k
                                    nc.tensor.matmul(
                                        out=agg[:],
                                        lhsT=sload[:, swi,
                                                   t * P : (t + 1) * P],
                                        rhs=wf[:, k, :],
                                        start=(t == 0),
                                        stop=(t == tn - 1),
                                    )
                            if half == "lo":
                                nc.scalar.copy(aggL[:, w, :], agg[:])
                                continue
                            # ---- window flush (hi pass) ----
                            tot = sb_pool.tile([P, P + 8], FP, tag="tot")
                            nc.vector.tensor_tensor(
                                out=tot[:], in0=agg[:], in1=aggL[:, w, :],
                                op=AO.add,
                            )
                            sguard = sb_pool.tile([P, 8], FP, tag="sguard")
                            nc.vector.tensor_scalar_max(
                                sguard[:], tot[:, P : P + 8], 1e-30
                            )
                            rec = sb_pool.tile([P, 8], FP, tag="rec")
                            nc.vector.reciprocal(rec[:], sguard[:])
                            o1 = sb_pool.tile([P, P], FP, tag="o1")
                            nc.vector.tensor_tensor(
                                out=o1[:].rearrange("p (h d) -> p h d", d=DH),
                                in0=tot[:, 0:P].rearrange(
                                    "p (h d) -> p h d", d=DH),
                                in1=rec[:].unsqueeze(2).to_broadcast(
                                    [P, HEADS, DH]),
                                op=AO.mult,
                            )
                            nc.vector.tensor_tensor(
                                out=o1[:], in0=o1[:], in1=hch[:, w, :],
                                op=AO.add
                            )
                            nc.scalar.activation(hnew[:, w, :], o1[:],
                                                 AFT.Relu)
                            flush_chunk(w, hnew, None, None,
                                        aginq_out, table8_out)
                            drain_ags(w)
                return hnew

            # =========================================================
            # forward pass
            # =========================================================
            gc_layer(0, tables[0], Wgc_sb[0], agin[0], tables[1],
                     scale_ns=True, two_pass=False)
            h2 = gc_layer(1, tables[1], Wgc_sb[1], None, None,
                          scale_ns=False, aginq_out=aginq[1],
                          table8_out=tables8[2])
            h3 = gat_layer(0, tables8[2], tables[2], h2, Ws_sb[0], Wd_sb[0],
                           abd_sb[0], aginq[2], tables8[3])
            h4 = gat_layer(1, tables8[3], tables[3], h3, Ws_sb[1], Wd_sb[1],
                           abd_sb[1], aginq[3], tables8[4])
            h5 = gat_layer(2, tables8[4], tables[4], h4, Ws_sb[2],
                           Wd_sb[2], abd_sb[2], None, None)

            # =========================================================
            # pooling + MLP (replicated)
            # =========================================================


            h5T = chunk_pool.tile([P, NW, P], BF, tag="hch")
            for w in range(NW):
                tp = mini_ps.tile([P, P], BF, tag="mini")
                nc.tensor.transpose(tp[:], h5[:, w, :], ident_bf[:])
                nc.scalar.copy(h5T[:, w, :], tp[:])

            NSEG = NW * KSEG
            stag = chunk_pool.tile([P, NSEG], FP, tag="stag")
            for w in range(NW):
                if w % 8 == 0:
                    nw8 = min(8, NW - w)
                    pmask_rep8 = sb_pool.tile(
                        [P, 8, KSEG * P], BF, tag="snT4", bufs=2,
                        name="pmask_rep8"
                    )
                    nc.sync.dma_start(
                        pmask_rep8[:, :nw8, :],
                        poolmask[w : w + nw8, :]
                        .unsqueeze(0)
                        .to_broadcast([P, nw8, KSEG * P]),
                    )
                pmask_rep = pmask_rep8[:, w % 8]
                msk = sb_pool.tile([P, KSEG, P], BF, tag="msk")
                nc.vector.tensor_tensor(
                    out=msk[:],
                    in0=h5T[:, w, :].unsqueeze(1).to_broadcast([P, KSEG, P]),
                    in1=pmask_rep[:].rearrange("p (k f) -> p k f", f=P),
                    op=AO.min,
                )
                nc.vector.tensor_reduce(
                    out=stag[:, w * KSEG : (w + 1) * KSEG], in_=msk[:],
                    axis=mybir.AxisListType.X, op=AO.max,
                )
            # graph-level masked max over segment columns -> hgT partial
            hgT_part = sb_pool.tile([P, G], FP, tag="hgT_part")
            gmask_all = sb_pool.tile([P, G, NSEG], BF, tag="dstrep4", bufs=1)
            nc.sync.dma_start(
                gmask_all[:],
                gmask[:].unsqueeze(0).to_broadcast([P, G, NSEG]),
            )
            GB = 16
            for g in range(0, G, GB):
                gm = sb_pool.tile([P, GB, NSEG], FP, tag="gm", bufs=2)
                nc.vector.tensor_tensor(
                    out=gm[:],
                    in0=stag[:, :NSEG].unsqueeze(1).to_broadcast([P, GB, NSEG]),
                    in1=gmask_all[:, g : g + GB], op=AO.min,
                )
                nc.vector.tensor_reduce(
                    out=hgT_part[:, g : g + GB], in_=gm[:],
                    axis=mybir.AxisListType.X, op=AO.max,
                )
            nc.sync.dma_start(hgpart[:], hgT_part[:])
            nc.gpsimd.collective_compute(
                "AllGather", AO.bypass, replica_groups=RG,
                ins=[hgpart.ap().opt()], outs=[hgall.ap().opt()],
            )
            # final max over ranks: hgall rows = (r p)
            hgl = sb_pool.tile([P, N_CORES * G], FP, tag="hgl")
            nc.sync.dma_start(
                hgl[:].rearrange("p (r g) -> p r g", g=G),
                hgall[:].rearrange("(r p) g -> p r g", p=P),
            )
            hgT = sb_pool.tile([P, G], FP, tag="hgT")
            nc.vector.tensor_reduce(
                out=hgT[:],
                in_=hgl[:].rearrange("p (r g) -> p g r", g=G),
                axis=mybir.AxisListType.X, op=AO.max,
            )

            Wc1_sb = load_const(Wc1, [P, P], FP)
            Wc2_sb = load_const(Wc2, [P, 64], FP)
            Wc3_sb = load_const(Wc3, [64, OUT], FP)

            z1p = mini_ps.tile([G, P], FP, tag="mini")
            nc.tensor.matmul(out=z1p[:], lhsT=hgT[:], rhs=Wc1_sb[:],
                             start=True, stop=True)
            z1 = sb_pool.tile([G, P], FP, tag="z1")
            nc.scalar.activation(z1[:], z1p[:], AFT.Relu)
            z1Tp = mini_ps.tile([P, G], FP, tag="mini")
            nc.tensor.transpose(z1Tp[:], z1[:], ident_f[:G, :G])
            z1T = sb_pool.tile([P, G], FP, tag="z1T")
            nc.scalar.copy(z1T[:], z1Tp[:])
            z2p = mini_ps.tile([G, 64], FP, tag="mini")
            nc.tensor.matmul(out=z2p[:], lhsT=z1T[:], rhs=Wc2_sb[:],
                             start=True, stop=True)
            z2 = sb_pool.tile([G, 64], FP, tag="z2")
            nc.scalar.activation(z2[:], z2p[:], AFT.Relu)
            z2Tp = mini_ps.tile([64, G], FP, tag="mini")
            nc.tensor.transpose(z2Tp[:], z2[:], ident_f[:G, :G])
            z2T = sb_pool.tile([64, G], FP, tag="z2T")
            nc.scalar.copy(z2T[:], z2Tp[:])
            z3p = mini_ps.tile([G, OUT], FP, tag="mini")
            nc.tensor.matmul(out=z3p[:], lhsT=z2T[:], rhs=Wc3_sb[:],
                             start=True, stop=True)
            z3 = sb_pool.tile([G, OUT], FP, tag="z3")
            nc.scalar.copy(z3[:], z3p[:])
            nc.sync.dma_start(out_ext[:], z3[:])

    nc.compile()
    return nc


# ---------------------------------------------------------------------------
# Entry point
# ---------------------------------------------------------------------------

def _run(inputs, nw_per_core=49, trace=False):
    from concourse.bass_utils import run_bass_kernel_spmd

    src = np.asarray(inputs["src"])
    dst = np.asarray(inputs["dst"])
    n2g = np.asarray(inputs["node2graph"])
    feat = np.asarray(inputs["feature"], np.float32)

    cfg, per_core, ns, nd, perm_src = prep(src, dst, n2g, nw_per_core)
    NP = cfg["NP"]

    featp = np.zeros((NP, P), np.float32)
    featp[: feat.shape[0]] = feat
    featp *= ns[:, None]
    table0 = np.ascontiguousarray(featp[perm_src]).astype(bf16)

    def b(x):
        return np.ascontiguousarray(np.asarray(x, np.float32).astype(bf16))

    SPLIT = cfg["SPLIT"]
    common = dict(
        table0lo=np.ascontiguousarray(table0[:SPLIT]),
        table0hi=np.ascontiguousarray(table0[SPLIT:]),
        Wgc0=b(inputs["W_gc1"]), Wgc1=b(inputs["W_gc2"]),
        Wc1=np.ascontiguousarray(np.asarray(inputs["Wc1"], np.float32)),
        Wc2=np.ascontiguousarray(np.asarray(inputs["Wc2"], np.float32)),
        Wc3=np.ascontiguousarray(np.asarray(inputs["Wc3"], np.float32)),
    )
    attn = np.asarray(inputs["attn"], np.float32)
    for i in range(3):
        common[f"Ws{i}"] = b(np.asarray(inputs["W_src"], np.float32)[i])
        common[f"Wd{i}"] = b(np.asarray(inputs["W_dst"], np.float32)[i])
        ar = np.zeros((P, HEADS), np.float32)
        for h in range(HEADS):
            ar[h * DH : (h + 1) * DH, h] = attn[i][h]
        common[f"abd{i}"] = np.ascontiguousarray(ar).astype(bf16)

    in_maps = []
    for c in range(N_CORES):
        m = dict(common)
        m.update(per_core[c])
        in_maps.append(m)

    nc = build_nc(cfg)
    res = run_bass_kernel_spmd(nc, in_maps, core_ids=list(range(N_CORES)),
                               trace=trace)
    return np.asarray(res.results[0]["out"], np.float32), res


def kernel(**inputs) -> np.ndarray:
    out, _ = _run(inputs)
    return out



# revision 53
# speedup vs baseline: 1.0496x; 1.0438x over previous
"""Trainium2 Bass kernel for nn_DifferentPooling (GNN message passing).

Strategy (8 NeuronCores, SPMD):
  - Nodes padded to NP = 8*CHUNK, partitioned by node id across cores in a
    K=4 chunk-permuted address space; edges partitioned by dst core and
    bucketed into 128-node dst windows. Aggregations (segment sum /
    softmax-sum) run on the TensorEngine as one-hot matmuls; the one-hot
    S_en / S_en^T matrices are built host-side and streamed from DRAM.
  - Per layer the 8 per-core feature chunks are AllGather'd (4 chunks,
    issued as producing windows finish). The GAT-feeding tables travel as
    fp8-e4m3 (half collective bytes on the serial collective device) and
    are upcast to bf16 gather tables on the consumer in partition-major
    stripes (DVE+Act split). The GC2 table is precision-critical and stays
    bf16.
  - Layers (except GC1, whose table is the kernel input) process all
    windows in two passes: lo-half edge tiles first (only needs the lo
    half of the incoming table, i.e. the first 2 AllGather chunks), then
    hi-half tiles + combine with the saved lo partials - overlapping each
    layer's start with its predecessor's trailing collectives.
  - GATv2: eps/leaky-relu computed feature-major (weight-stationary
    matmuls, N=512), logits via a block-diagonal attention matmul
    (edge-major out), exp on Act; alpha-weighting and the [num | den]
    aggregation stay edge-major with den in 8 extra PSUM columns.
  - Softmax uses exp(logit) without max subtraction (logits tiny) with a
    1e-30 guard; graph max-pooling via masked-max segments and a small
    AllGather; replicated fp32 MLP head.

All biases in this problem are zeros by spec (fill="zeros"); they are not
applied on device.
"""

import sys

sys.path.insert(0, "/opt/trn_rl_repo")

import numpy as np
import ml_dtypes

bf16 = ml_dtypes.bfloat16
fp8 = ml_dtypes.float8_e4m3fn

N_CORES = 8
P = 128  # window size / partition count
N_REAL = 50000
E_REAL = 500000
G = 64
HID = 128
HEADS = 8
DH = 16
OUT = 256


# ---------------------------------------------------------------------------
# Host-side preprocessing
# ---------------------------------------------------------------------------

def _wrap_idx(arr):
    """int idx array (len % 16 == 0) -> [128, len/16] int16 wrapped layout:
    idx i lives at [i % 16, i // 16], replicated across the 8 groups of 16
    partitions (one per Q7 core)."""
    a = np.asarray(arr, np.int16).reshape(-1, 16).T  # [16, cols]
    return np.tile(a, (8, 1))  # [128, cols]


def prep(src, dst, node2graph, nw_per_core, kchunks=4):
    """Build per-core edge/window metadata. Returns (cfg, host arrays).

    The node table on device lives in a chunk-permuted layout so each
    layer's AllGather can be split into `kchunks` window-chunks issued as
    soon as the producing windows finish: address order is
    [chunk][rank][window-in-chunk][pos], matching what chunked AllGathers
    over agin row-slices naturally produce. All gather indices below are
    built in permuted address space.
    """
    NW = nw_per_core
    CHUNK = NW * P
    NP = N_CORES * CHUNK
    HALF = NP // 2
    N = len(node2graph)
    E = len(src)

    src = np.asarray(src, np.int64)
    dst = np.asarray(dst, np.int64)
    n2g = np.asarray(node2graph, np.int64)

    # chunk structure (in windows): even split measured best (smaller-first
    # and K=5 both regressed against the cost model's 15us/chunk overhead).
    K = kchunks
    wc = [NW // K] * K
    for c in range(NW - sum(wc)):
        wc[c] += 1
    w0s = np.cumsum([0] + wc)[:-1]
    chunk_of_win = np.repeat(np.arange(K), wc)
    cbase = N_CORES * P * np.cumsum([0] + wc)[:-1]
    # lo/hi gather split at the chunk K/2 boundary so the lo and hi table
    # halves can live in separate DRAM tensors (precise collective deps)
    SPLIT = int(cbase[K // 2]) if K >= 2 else HALF

    def paddr(n):
        r, local = np.divmod(np.asarray(n, np.int64), CHUNK)
        w, p = np.divmod(local, P)
        c = chunk_of_win[w]
        return cbase[c] + (r * np.asarray(wc)[c] + (w - w0s[c])) * P + p

    outdeg = np.zeros(NP, np.float32)
    np.add.at(outdeg, src, 1.0)
    indeg = np.zeros(NP, np.float32)
    np.add.at(indeg, dst, 1.0)
    ns = np.maximum(outdeg, 1.0) ** -0.5
    nd = np.maximum(indeg, 1.0) ** -0.5

    # sort edges by dst, bucket into windows; src ids move to permuted space
    order = np.argsort(dst, kind="stable")
    sdst = dst[order]
    ssrc = paddr(src[order])
    n_win_total = NP // P
    win_starts = np.searchsorted(sdst, np.arange(0, NP + 1, P))

    # per (global window): lo/hi edge lists sorted by src
    lo_lists, hi_lists = [], []
    max_lo = max_hi = 1
    for w in range(n_win_total):
        a, b = win_starts[w], win_starts[w + 1]
        es, ed = ssrc[a:b], sdst[a:b] - w * P
        m = es < SPLIT
        ordl = np.argsort(es[m], kind="stable")
        ordh = np.argsort(es[~m], kind="stable")
        lo_lists.append((es[m][ordl], ed[m][ordl]))
        hi_lists.append((es[~m][ordh] - SPLIT, ed[~m][ordh]))
        max_lo = max(max_lo, len(lo_lists[-1][0]))
        max_hi = max(max_hi, len(hi_lists[-1][0]))

    L = (max_lo + P - 1) // P
    H = (max_hi + P - 1) // P
    T = L + H

    # spans of SPAN_W windows (gather batching granularity)
    SPAN_W = 4 if NW >= 8 else 2
    spans = []
    w0 = 0
    while w0 < NW:
        spans.append((w0, min(SPAN_W, NW - w0)))
        w0 += SPAN_W

    per_core = []
    for c in range(N_CORES):
        idx_lo = np.zeros((NW, L * P), np.int64)
        dst_lo = np.full((NW, L * P), P, np.int64)  # sentinel 128
        idx_hi = np.zeros((NW, H * P), np.int64)
        dst_hi = np.full((NW, H * P), P, np.int64)
        for w in range(NW):
            el, dl = lo_lists[c * NW + w]
            eh, dh_ = hi_lists[c * NW + w]
            idx_lo[w, : len(el)] = el
            dst_lo[w, : len(dl)] = dl
            idx_hi[w, : len(eh)] = eh
            dst_hi[w, : len(dh_)] = dh_
        # dstloc: [NW*T, 128] -> transpose to [128, NW*T]; col w*T+t
        dstloc = np.concatenate(
            [dst_lo.reshape(NW, L, P), dst_hi.reshape(NW, H, P)], axis=1
        ).reshape(NW * T, P)
        ndw = nd[c * CHUNK : (c + 1) * CHUNK].reshape(NW, P).T.copy()
        nsw = ns[c * CHUNK : (c + 1) * CHUNK].reshape(NW, P).T.copy()
        # host-built one-hot selection matrices:
        # sden[w, e, t*128+n] = (dst-pos of edge slot (w,t,e) == n)
        # snden[w, p, t*128+e] = sden[w, e, t*128+p]  (transpose)
        eye = np.zeros((P + 1, P), bf16)
        eye[np.arange(P), np.arange(P)] = 1
        oh = eye[dstloc]                       # [NW*T, P(e), P(n)]
        sden_h = np.ascontiguousarray(
            oh.reshape(NW, T, P, P).transpose(0, 2, 1, 3)
            .reshape(NW, P, T * P))
        snden_h = np.ascontiguousarray(
            oh.reshape(NW, T, P, P).transpose(0, 3, 1, 2)
            .reshape(NW, P, T * P))
        per_core.append(
            dict(
                idx_lo=_wrap_idx(idx_lo.reshape(-1)),
                idx_hi=_wrap_idx(idx_hi.reshape(-1)),
                sden=sden_h,
                sden8=sden_h.astype(fp8),
                snden=snden_h.astype(fp8),
                ndw=np.ascontiguousarray(ndw, np.float32),
                nsw=np.ascontiguousarray(nsw, np.float32),
            )
        )

    # pooling segments per core: runs of equal graph id inside each window
    n2g_pad = np.full(NP, -1, np.int64)
    n2g_pad[:N] = n2g
    seg_all = []  # per core: list of (w, j0, j1, g)
    KSEG = 1
    for c in range(N_CORES):
        segs = []
        for w in range(NW):
            ids = n2g_pad[c * CHUNK + w * P : c * CHUNK + (w + 1) * P]
            j = 0
            wsegs = []
            while j < P:
                g = ids[j]
                k = j
                while k < P and ids[k] == g:
                    k += 1
                if g >= 0:
                    wsegs.append((j, k, int(g)))
                j = k
            KSEG = max(KSEG, len(wsegs))
            segs.append(wsegs)
        seg_all.append(segs)

    BIG = np.float32(1e30)
    NSEG = NW * KSEG
    for c in range(N_CORES):
        maskvec = np.full((NW, KSEG, P), -BIG, np.float32)
        gmask = np.full((G, NSEG), -BIG, np.float32)
        for w in range(NW):
            for k, (j0, j1, g) in enumerate(seg_all[c][w]):
                maskvec[w, k, j0:j1] = BIG
                gmask[g, w * KSEG + k] = BIG
        per_core[c]["poolmask"] = maskvec.reshape(NW, KSEG * P).astype(bf16)
        per_core[c]["gmask"] = gmask.astype(bf16)

    cfg = dict(NW=NW, CHUNK=CHUNK, NP=NP, HALF=HALF, SPLIT=SPLIT,
               L=L, H=H, T=T, spans=spans, KSEG=KSEG,
               chunks=list(zip(w0s.tolist(), wc)))
    perm_src = np.empty(NP, np.int64)  # perm_src[paddr] = original id
    perm_src[paddr(np.arange(NP))] = np.arange(NP)
    return cfg, per_core, ns, nd, perm_src


# ---------------------------------------------------------------------------
# Bass kernel builder
# ---------------------------------------------------------------------------

def build_nc(cfg):
    import concourse.bacc as bacc
    import concourse.bass as bass
    import concourse.mybir as mybir
    import concourse.tile as tile
    from concourse.masks import make_identity

    NW, CHUNK, NP, HALF = cfg["NW"], cfg["CHUNK"], cfg["NP"], cfg["HALF"]
    L, H, T, spans, KSEG = cfg["L"], cfg["H"], cfg["T"], cfg["spans"], cfg["KSEG"]
    chunks = cfg["chunks"]
    SPLIT = cfg["SPLIT"]
    FP = mybir.dt.float32
    BF = mybir.dt.bfloat16
    F8 = mybir.dt.float8e4
    AO = mybir.AluOpType
    AFT = mybir.ActivationFunctionType

    nc = bacc.Bacc("TRN2", target_bir_lowering=False, debug=False,
                   num_devices=N_CORES)

    def din(name, shape, dt=BF):
        return nc.dram_tensor(name, shape, dt, kind="ExternalInput")

    table0 = (din("table0lo", [SPLIT, P]), din("table0hi", [NP - SPLIT, P]))
    Wgc = [din(f"Wgc{i}", [P, P]) for i in range(2)]
    Ws = [din(f"Ws{i}", [P, P]) for i in range(3)]
    Wd = [din(f"Wd{i}", [P, P]) for i in range(3)]
    abd = [din(f"abd{i}", [P, HEADS]) for i in range(3)]
    Wc1 = din("Wc1", [P, P], FP)
    Wc2 = din("Wc2", [P, 64], FP)
    Wc3 = din("Wc3", [64, OUT], FP)
    idx_lo = din("idx_lo", [P, NW * L * P // 16], mybir.dt.int16)
    idx_hi = din("idx_hi", [P, NW * H * P // 16], mybir.dt.int16)
    sden = din("sden", [NW, P, T * P])
    sden8 = din("sden8", [NW, P, T * P], F8)
    snden = din("snden", [NW, P, T * P], F8)
    ndw = din("ndw", [P, NW], FP)
    nsw = din("nsw", [P, NW], FP)
    poolmask = din("poolmask", [NW, KSEG * P])
    gmask = din("gmask", [G, NW * KSEG])

    out_ext = nc.dram_tensor("out", [G, OUT], FP, kind="ExternalOutput")

    # internal DRAM
    # agin: bf16 own-chunk copies (hch source); aginq: fp8 collective inputs
    # for the GAT-feeding layers (tables 2..4 travel as fp8 and are upcast
    # to bf16 gather tables on the consumer side).
    agin = [nc.dram_tensor("agin0", [CHUNK, P], BF)]
    aginq = [nc.dram_tensor(f"aginq{i}", [CHUNK, P], F8) for i in range(4)]
    tables = [table0,
              (nc.dram_tensor("tlo1", [SPLIT, P], BF, addr_space="Shared"),
               nc.dram_tensor("thi1", [NP - SPLIT, P], BF,
                              addr_space="Shared"))]
    rc = [wcc * P * N_CORES for (_w0c, wcc) in chunks]
    tables8 = [None, None]
    for i in range(2, 5):
        tables8.append(tuple(
            nc.dram_tensor(f"t8c{i}_{k}", [rc[k], P], F8,
                           addr_space="Shared")
            for k in range(len(chunks))))
        tables.append(
            (nc.dram_tensor(f"tlo{i}", [SPLIT, P], BF),
             nc.dram_tensor(f"thi{i}", [NP - SPLIT, P], BF)))
    hgpart = nc.dram_tensor("hgpart", [P, G], FP)
    hgall = nc.dram_tensor("hgall", [N_CORES * P, G], FP, addr_space="Shared")

    RG = [list(range(N_CORES))]

    with tile.TileContext(nc) as tc:
        import contextlib

        ctx = contextlib.ExitStack()
        with ctx:
            const_pool = ctx.enter_context(tc.tile_pool(name="const", bufs=1))
            stg_pool = ctx.enter_context(tc.tile_pool(name="stg", bufs=2))
            sb_pool = ctx.enter_context(tc.tile_pool(name="sb", bufs=3))
            chunk_pool = ctx.enter_context(tc.tile_pool(name="chunk", bufs=1))
            ps_pool = ctx.enter_context(
                tc.tile_pool(name="ps", bufs=2, space="PSUM")
            )
            snt_pool = ctx.enter_context(
                tc.tile_pool(name="snt", bufs=1, space="PSUM")
            )
            agg_pool = ctx.enter_context(
                tc.tile_pool(name="agg", bufs=2, space="PSUM")
            )
            mini_ps = ctx.enter_context(
                tc.tile_pool(name="minips", bufs=2, space="PSUM")
            )

            # --- constants in SBUF ---
            ident_bf = const_pool.tile([P, P], BF, tag="identbf")
            make_identity(nc, ident_bf[:])
            ident_f = const_pool.tile([P, P], FP, tag="identf")
            make_identity(nc, ident_f[:])

            def load_const(h, shape, dt=BF, tag=None):
                t = const_pool.tile(shape, dt, tag=tag or h.name)
                nc.sync.dma_start(t[:], h[:])
                return t

            Wgc_sb = [load_const(w, [P, P]) for w in Wgc]
            Ws_sb = [load_const(w, [P, P]) for w in Ws]
            Wd_sb = [load_const(w, [P, P]) for w in Wd]
            abd_sb = [load_const(w, [P, HEADS]) for w in abd]
            ndw_sb = load_const(ndw, [P, NW], FP)
            nsw_sb = load_const(nsw, [P, NW], FP)
            idxlo_sb = load_const(idx_lo, [P, NW * L * P // 16], mybir.dt.int16)
            idxhi_sb = load_const(idx_hi, [P, NW * H * P // 16], mybir.dt.int16)

            SLOAD_W = 4  # windows per S_en reload DMA

            def s_en_load(w0, nwin, t0, tn, src_t=None, dt=None, tag="sload"):
                """Load S_en tiles [t0, t0+tn) for windows [w0, w0+nwin)."""
                if src_t is None:
                    src_t, dt = sden, BF
                sload = sb_pool.tile([P, SLOAD_W, max(L, H) * P], dt,
                                     tag=tag, bufs=2)
                nc.sync.dma_start(
                    sload[:, :nwin, : tn * P],
                    src_t[w0 : w0 + nwin, :,
                          t0 * P : (t0 + tn) * P].rearrange("w p f -> p w f"),
                )
                return sload

            def gather_span(table_l, w0, nw, transpose, which):
                """Gather the lo or hi edges of windows [w0, w0+nw).
                transpose -> [128, 1, n] column tiles, else
                [128, ntiles, 128] row tiles."""
                if which == "lo":
                    n, idx_sb, colpos = nw * L * P, idxlo_sb, w0 * L * P
                    half = table_l[0][0:SPLIT, :]
                else:
                    n, idx_sb, colpos = nw * H * P, idxhi_sb, w0 * H * P
                    half = table_l[1][0 : NP - SPLIT, :]
                nmax = nw * max(L, H) * P
                if transpose:
                    t = stg_pool.tile([P, 1, nmax], BF, tag="stg", bufs=3)
                    t = t[:, :, :n]
                else:
                    t = stg_pool.tile([P, nmax // P, P], BF, tag="stg", bufs=3)
                    t = t[:, : n // P, :]
                nc.gpsimd.dma_gather(
                    t[:, :, :],
                    half,
                    idx_sb[:, colpos // 16 : (colpos + n) // 16],
                    n,
                    n,
                    P,
                    transpose=transpose,
                    single_packet=False,
                )
                return t

            chunk_end = {w0c + wcc - 1: (w0c, wcc) for (w0c, wcc) in chunks}

            def flush_chunk(w, hnew, agin_out, table_out, aginq_out=None,
                            table8_out=None):
                """After window w completes, DMA the finished chunk's rows
                out (bf16 agin copy and/or fp8 aginq). The AllGather itself
                is emitted a few windows LATER (see drain_ags): collectives
                issue from the Pool sequencer, and a collective emitted
                right at chunk completion holds Pool.SEQ waiting for the
                chunk DMA - blocking the remaining span gathers behind it.
                Delaying emission lets it issue with its input already in
                DRAM (short SEQ hold) while still starting early."""
                if w not in chunk_end:
                    return
                if aginq_out is None and agin_out is None:
                    return
                w0c, wcc = chunk_end[w]
                r0, r1 = w0c * P, (w0c + wcc) * P
                g0 = r0 * N_CORES
                kc = [i for i, (a, _b) in enumerate(chunks) if a == w0c][0]
                emit_w = min(w + 3, NW - 1)
                if table8_out is not None:
                    h8 = sb_pool.tile([P, (NW + len(chunks) - 1)
                                       // len(chunks), P], F8, tag="h8",
                                      bufs=2)
                    nc.vector.tensor_copy(
                        h8[:, :wcc, :], hnew[:, w0c : w0c + wcc, :])
                    nc.sync.dma_start(
                        aginq_out[r0:r1, :].rearrange("(w p) f -> p w f", p=P),
                        h8[:, :wcc, :],
                    )
                    pending_ags.setdefault(emit_w, []).append(
                        lambda: nc.gpsimd.collective_compute(
                            "AllGather", AO.bypass, replica_groups=RG,
                            ins=[aginq_out[r0:r1, :].opt()],
                            outs=[table8_out[kc][:, :].opt()],
                        ))
                elif table_out is not None:
                    nc.sync.dma_start(
                        agin_out[r0:r1, :].rearrange("(w p) f -> p w f", p=P),
                        hnew[:, w0c : w0c + wcc, :],
                    )
                    if g0 < SPLIT:
                        tgt, off = table_out[0], g0
                    else:
                        tgt, off = table_out[1], g0 - SPLIT
                    pending_ags.setdefault(emit_w, []).append(
                        lambda tgt=tgt, off=off: nc.gpsimd.collective_compute(
                            "AllGather", AO.bypass, replica_groups=RG,
                            ins=[agin_out[r0:r1, :].opt()],
                            outs=[tgt[off : off + (r1 - r0)
                                      * N_CORES, :].opt()],
                        ))

            pending_ags = {}

            def drain_ags(w):
                for fn in pending_ags.pop(w, []):
                    fn()

            # =========================================================
            # GraphConv layers
            # =========================================================
            def gc_layer(li, table_l, W_sb, agin_out, table_out, scale_ns,
                         aginq_out=None, table8_out=None, two_pass=True,
                         table8_l=None):
                """two_pass: lo-half tiles for all windows first (only needs
                the lo table chunks), then hi-half tiles + combine — lets
                this layer start before its hi table arrives. GC1's table is
                an input (no collective), so it runs single-pass for earlier
                chunk completion."""
                hnew = chunk_pool.tile([P, NW, P], BF, tag="hnew")
                aggLg = chunk_pool.tile([P, NW, P + 8], BF, tag="aggLg")
                aggL = aggLg[:, :, :P]
                if two_pass:
                    halves = (("lo", (0, L)), ("hi", (L, H)))
                else:
                    halves = (("all", (0, T)),)
                for half, (t0, tn) in halves:
                    if table8_l is not None:
                        if half in ("lo", "all"):
                            upcast_half(table8_l, table_l, "lo")
                        if half in ("hi", "all"):
                            upcast_half(table8_l, table_l, "hi")
                    for (w0, nw) in spans:
                        if half == "all":
                            stg_lo = gather_span(table_l, w0, nw, False, "lo")
                            stg_hi = gather_span(table_l, w0, nw, False, "hi")
                        else:
                            stg = gather_span(table_l, w0, nw, False, half)
                        for wr in range(nw):
                            w = w0 + wr
                            if half == "all":
                                if (w - w0) % SLOAD_W == 0:
                                    sl_lo = s_en_load(
                                        w, min(SLOAD_W, nw - wr), 0, L)
                                    sl_hi = s_en_load(
                                        w, min(SLOAD_W, nw - wr), L, H)
                            else:
                                if (w - w0) % SLOAD_W == 0:
                                    sload = s_en_load(
                                        w, min(SLOAD_W, nw - wr), t0, tn)
                            swi = (w - w0) % SLOAD_W
                            aggT_full = agg_pool.tile([P, P + 8], FP,
                                                      tag="agg", name="aggT")
                            aggT = aggT_full[:, :P]
                            for k in range(tn):
                                if half == "all":
                                    if k < L:
                                        lhs = stg_lo[:, wr * L + k, :]
                                        sen = sl_lo[:, swi,
                                                    k * P : (k + 1) * P]
                                    else:
                                        lhs = stg_hi[:, wr * H + (k - L), :]
                                        sen = sl_hi[:, swi, (k - L) * P
                                                    : (k - L + 1) * P]
                                else:
                                    lhs = stg[:, wr * tn + k, :]
                                    sen = sload[:, swi, k * P : (k + 1) * P]
                                nc.tensor.matmul(
                                    out=aggT[:],
                                    lhsT=lhs,
                                    rhs=sen,
                                    start=(k == 0),
                                    stop=(k == tn - 1),
                                )
                            if half == "lo":
                                nc.scalar.copy(aggL[:, w, :], aggT[:])
                                continue
                            aggT_sb = sb_pool.tile([P, P], BF, tag="aggTsb")
                            if half == "hi":
                                nc.vector.tensor_tensor(
                                    out=aggT_sb[:], in0=aggT[:],
                                    in1=aggL[:, w, :], op=AO.add,
                                )
                            else:
                                nc.scalar.copy(aggT_sb[:], aggT[:])
                            op = mini_ps.tile([P, P], FP, tag="mini")
                            nc.tensor.matmul(out=op[:], lhsT=aggT_sb[:],
                                             rhs=W_sb[:],
                                             start=True, stop=True)
                            nc.scalar.activation(
                                hnew[:, w, :], op[:], AFT.Relu,
                                scale=ndw_sb[:, w : w + 1],
                            )
                            if scale_ns:
                                nc.vector.tensor_scalar_mul(
                                    hnew[:, w, :], hnew[:, w, :],
                                    nsw_sb[:, w : w + 1],
                                )
                            flush_chunk(w, hnew, agin_out, table_out,
                                        aginq_out, table8_out)
                            drain_ags(w)
                return hnew

            # =========================================================
            # GATv2 layers
            # =========================================================
            def upcast_half(table8_l, table_l, which):
                """fp8 per-chunk tables -> bf16 gather half-table, in
                partition-major contiguous stripes (big DMA descriptors,
                cheap convert split across DVE and Act). Per-chunk source
                tensors let each upcast start as its AllGather lands."""
                nk = len(chunks)
                ks = tuple(range(nk // 2)) if which == "lo" else tuple(range(nk // 2, nk))
                tbhalf = table_l[0] if which == "lo" else table_l[1]
                base = 0
                for k in ks:
                    nrows = rc[k]
                    npiece = 2
                    rows = nrows // npiece
                    assert rows * npiece == nrows and rows % P == 0
                    for i in range(npiece):
                        per = rows  # elems per partition
                        pmax = max(rc) // 2
                        s8 = sb_pool.tile([P, pmax], F8, tag="upc8", bufs=2)
                        nc.sync.dma_start(
                            s8[:, :per],
                            table8_l[k][i * rows : (i + 1) * rows, :]
                            .rearrange("(p a) f -> p (a f)", p=P),
                        )
                        sb = sb_pool.tile([P, pmax], BF, tag="upcb", bufs=1)
                        hh = per // 2
                        nc.vector.tensor_copy(sb[:, :hh], s8[:, :hh])
                        nc.scalar.copy(sb[:, hh:per], s8[:, hh:per])
                        r0 = base + i * rows
                        nc.sync.dma_start(
                            tbhalf[r0 : r0 + rows, :].rearrange(
                                "(p a) f -> p (a f)", p=P),
                            sb[:, :per],
                        )
                    base += nrows

            def gat_layer(li, table8_l, table_l, h_prev, Ws_l, Wd_l,
                          abd_l, aginq_out, table8_out):
                # own chunk: SBUF copy of the previous layer's output tile
                # (made before this layer's hnew reuses that buffer)
                hch = chunk_pool.tile([P, NW, P], BF, tag="hch")
                nc.vector.tensor_copy(hch[:], h_prev[:])
                fdw = chunk_pool.tile([P, NW, P], F8, tag="fdw")
                for w in range(NW):
                    tp = mini_ps.tile([P, P], BF, tag="mini")
                    nc.tensor.transpose(tp[:], hch[:, w, :], ident_bf[:])
                    hwT = sb_pool.tile([P, P], BF, tag="hwTsb")
                    nc.scalar.copy(hwT[:], tp[:])
                    fp = mini_ps.tile([P, P], FP, tag="mini")
                    nc.tensor.matmul(out=fp[:], lhsT=hwT[:], rhs=Wd_l[:],
                                     start=True, stop=True)
                    nc.scalar.copy(fdw[:, w, :], fp[:])

                hnew = chunk_pool.tile([P, NW, P], BF, tag="hnew")
                aggL = chunk_pool.tile([P, NW, P + 8], BF, tag="aggLg")
                TM = max(L, H)
                for half, (t0, tn) in (("lo", (0, L)), ("hi", (L, H))):
                    upcast_half(table8_l, table_l, half)
                    for (w0, nw) in spans:
                        stg = gather_span(table_l, w0, nw, True, half)
                        for wr in range(nw):
                            w = w0 + wr
                            if wr % 4 == 0:
                                n4 = min(4, nw - wr)
                                snT4 = sb_pool.tile([P, 4, TM, P], F8,
                                                    tag="snT48", bufs=2,
                                                    name="snT4")
                                nc.sync.dma_start(
                                    snT4[:, :n4, :tn, :].rearrange(
                                        "p w t f -> p w (t f)"),
                                    snden[w : w + n4, :,
                                          t0 * P : (t0 + tn) * P].rearrange(
                                        "w p f -> p w f"),
                                )
                            snTw = snT4[:, wr % 4]
                            if (w - w0) % SLOAD_W == 0:
                                sload = s_en_load(w, min(SLOAD_W, nw - wr),
                                                  t0, tn, sden8, F8,
                                                  tag="sload8")
                            swi = (w - w0) % SLOAD_W
                            agg = agg_pool.tile([P, P + 8], FP, tag="agg")

                            # phase 1 (feature-major): epsf[f', e] = Ws^T @
                            # h_cols + fdw^T @ snT in <=4-tile runs, prelu'd
                            # into elrf (still feature-major).
                            elrf = sb_pool.tile([P, TM, P], BF, tag="elr",
                                                bufs=2)
                            groups = []
                            g0 = 0
                            while g0 < tn:
                                gn = min(4, tn - g0)
                                groups.append((g0, gn))
                                g0 += gn
                            for gi, (g0, gn) in enumerate(groups):
                                epsf = ps_pool.tile([P, 4 * P], FP,
                                                    tag=f"eps{gi % 2}",
                                                    bufs=1)
                                col = (wr * tn + g0) * P
                                nc.tensor.matmul(
                                    out=epsf[:, : gn * P], lhsT=Ws_l[:],
                                    rhs=stg[:, 0, col : col + gn * P],
                                    start=True, stop=False)
                                nc.tensor.matmul(
                                    out=epsf[:, : gn * P], lhsT=fdw[:, w, :],
                                    rhs=snTw[:, g0 : g0 + gn, :].rearrange(
                                        "p a b -> p (a b)"),
                                    start=False, stop=True)
                                nc.scalar.activation(
                                    elrf[:, g0 : g0 + gn, :],
                                    epsf[:, : gn * P].rearrange(
                                        "p (a b) -> p a b", b=P),
                                    AFT.Prelu, alpha=0.2,
                                )
                            # phase 2: per-tile logits via block-diag
                            # attention matmul (edge-major out), then exp
                            logps = mini_ps.tile([P, TM * HEADS], FP,
                                                 tag="mini")
                            for k in range(tn):
                                nc.tensor.matmul(
                                    out=logps[:, k * HEADS : (k + 1) * HEADS],
                                    lhsT=elrf[:, k, :], rhs=abd_l[:],
                                    start=True, stop=True)
                            pex = sb_pool.tile([P, TM * HEADS], BF, tag="pex")
                            nc.scalar.activation(pex[:, : tn * HEADS],
                                                 logps[:, : tn * HEADS],
                                                 AFT.Exp)
                            # phase 3: fs recompute + alpha-weighting + agg
                            for gi, (g0, gn) in enumerate(groups):
                                fsps = ps_pool.tile([P, 4 * P], FP,
                                                    tag=f"fsps{gi % 2}",
                                                    bufs=1)
                                for k in range(gn):
                                    col = (wr * tn + g0 + k) * P
                                    nc.tensor.matmul(
                                        out=fsps[:, k * P : (k + 1) * P],
                                        lhsT=stg[:, 0, col : col + P],
                                        rhs=Ws_l[:], start=True, stop=True)
                                wf = sb_pool.tile([P, 4, P + 8], F8, tag="wf")
                                nc.vector.tensor_copy(
                                    wf[:, :gn, P : P + 8],
                                    pex[:, g0 * HEADS : (g0 + gn) * HEADS]
                                    .rearrange("p (a b) -> p a b", b=HEADS),
                                )
                                nc.vector.tensor_tensor(
                                    out=wf[:, :gn, 0:P].rearrange(
                                        "p a (h d) -> p a h d", d=DH
                                    ),
                                    in0=fsps[:, : gn * P].rearrange(
                                        "p (a h d) -> p a h d", h=HEADS, d=DH
                                    ),
                                    in1=pex[:, g0 * HEADS : (g0 + gn) * HEADS]
                                    .rearrange("p (a h) -> p a h", h=HEADS)
                                    .unsqueeze(3)
                                    .to_broadcast([P, gn, HEADS, DH]),
                                    op=AO.mult,
                                )
                                for k in range(gn):
                                    t = g0 + k
                                    nc.tensor.matmul(
                                        out=agg[:],
                                        lhsT=sload[:, swi,
                                                   t * P : (t + 1) * P],
                                        rhs=wf[:, k, :],
                                        start=(t == 0),
                                        stop=(t == tn - 1),
                                    )
                            if half == "lo":
                                nc.scalar.copy(aggL[:, w, :], agg[:])
                                continue
                            # ---- window flush (hi pass) ----
                            tot = sb_pool.tile([P, P + 8], FP, tag="tot")
                            nc.vector.tensor_tensor(
                                out=tot[:], in0=agg[:], in1=aggL[:, w, :],
                                op=AO.add,
                            )
                            sguard = sb_pool.tile([P, 8], FP, tag="sguard")
                            nc.vector.tensor_scalar_max(
                                sguard[:], tot[:, P : P + 8], 1e-30
                            )
                            rec = sb_pool.tile([P, 8], FP, tag="rec")
                            nc.vector.reciprocal(rec[:], sguard[:])
                            o1 = sb_pool.tile([P, P], FP, tag="o1")
                            nc.vector.tensor_tensor(
                                out=o1[:].rearrange("p (h d) -> p h d", d=DH),
                                in0=tot[:, 0:P].rearrange(
                                    "p (h d) -> p h d", d=DH),
                                in1=rec[:].unsqueeze(2).to_broadcast(
                                    [P, HEADS, DH]),
                                op=AO.mult,
                            )
                            nc.vector.tensor_tensor(
                                out=o1[:], in0=o1[:], in1=hch[:, w, :],
                                op=AO.add
                            )
                            nc.scalar.activation(hnew[:, w, :], o1[:],
                                                 AFT.Relu)
                            flush_chunk(w, hnew, None, None,
                                        aginq_out, table8_out)
                            drain_ags(w)
                return hnew

            # =========================================================
            # forward pass
            # =========================================================
            gc_layer(0, tables[0], Wgc_sb[0], agin[0], tables[1],
                     scale_ns=True, two_pass=False)
            h2 = gc_layer(1, tables[1], Wgc_sb[1], None, None,
                          scale_ns=False, aginq_out=aginq[1],
                          table8_out=tables8[2])
            h3 = gat_layer(0, tables8[2], tables[2], h2, Ws_sb[0], Wd_sb[0],
                           abd_sb[0], aginq[2], tables8[3])
            h4 = gat_layer(1, tables8[3], tables[3], h3, Ws_sb[1], Wd_sb[1],
                           abd_sb[1], aginq[3], tables8[4])
            h5 = gat_layer(2, tables8[4], tables[4], h4, Ws_sb[2],
                           Wd_sb[2], abd_sb[2], None, None)

            # =========================================================
            # pooling + MLP (replicated)
            # =========================================================


            h5T = chunk_pool.tile([P, NW, P], BF, tag="hch")
            for w in range(NW):
                tp = mini_ps.tile([P, P], BF, tag="mini")
                nc.tensor.transpose(tp[:], h5[:, w, :], ident_bf[:])
                nc.scalar.copy(h5T[:, w, :], tp[:])

            NSEG = NW * KSEG
            stag = chunk_pool.tile([P, NSEG], FP, tag="stag")
            for w in range(NW):
                if w % 8 == 0:
                    nw8 = min(8, NW - w)
                    pmask_rep8 = sb_pool.tile(
                        [P, 8, KSEG * P], BF, tag="snT4", bufs=2,
                        name="pmask_rep8"
                    )
                    nc.sync.dma_start(
                        pmask_rep8[:, :nw8, :],
                        poolmask[w : w + nw8, :]
                        .unsqueeze(0)
                        .to_broadcast([P, nw8, KSEG * P]),
                    )
                pmask_rep = pmask_rep8[:, w % 8]
                msk = sb_pool.tile([P, KSEG, P], BF, tag="msk")
                nc.vector.tensor_tensor(
                    out=msk[:],
                    in0=h5T[:, w, :].unsqueeze(1).to_broadcast([P, KSEG, P]),
                    in1=pmask_rep[:].rearrange("p (k f) -> p k f", f=P),
                    op=AO.min,
                )
                nc.vector.tensor_reduce(
                    out=stag[:, w * KSEG : (w + 1) * KSEG], in_=msk[:],
                    axis=mybir.AxisListType.X, op=AO.max,
                )
            # graph-level masked max over segment columns -> hgT partial
            hgT_part = sb_pool.tile([P, G], FP, tag="hgT_part")
            gmask_all = sb_pool.tile([P, G, NSEG], BF, tag="dstrep4", bufs=1)
            nc.sync.dma_start(
                gmask_all[:],
                gmask[:].unsqueeze(0).to_broadcast([P, G, NSEG]),
            )
            GB = 16
            for g in range(0, G, GB):
                gm = sb_pool.tile([P, GB, NSEG], FP, tag="gm", bufs=2)
                nc.vector.tensor_tensor(
                    out=gm[:],
                    in0=stag[:, :NSEG].unsqueeze(1).to_broadcast([P, GB, NSEG]),
                    in1=gmask_all[:, g : g + GB], op=AO.min,
                )
                nc.vector.tensor_reduce(
                    out=hgT_part[:, g : g + GB], in_=gm[:],
                    axis=mybir.AxisListType.X, op=AO.max,
                )
            nc.sync.dma_start(hgpart[:], hgT_part[:])
            nc.gpsimd.collective_compute(
                "AllGather", AO.bypass, replica_groups=RG,
                ins=[hgpart.ap().opt()], outs=[hgall.ap().opt()],
            )
            # final max over ranks: hgall rows = (r p)
            hgl = sb_pool.tile([P, N_CORES * G], FP, tag="hgl")
            nc.sync.dma_start(
                hgl[:].rearrange("p (r g) -> p r g", g=G),
                hgall[:].rearrange("(r p) g -> p r g", p=P),
            )
            hgT = sb_pool.tile([P, G], FP, tag="hgT")
            nc.vector.tensor_reduce(
                out=hgT[:],
                in_=hgl[:].rearrange("p (r g) -> p g r", g=G),
                axis=mybir.AxisListType.X, op=AO.max,
            )

            Wc1_sb = load_const(Wc1, [P, P], FP)
            Wc2_sb = load_const(Wc2, [P, 64], FP)
            Wc3_sb = load_const(Wc3, [64, OUT], FP)

            z1p = mini_ps.tile([G, P], FP, tag="mini")
            nc.tensor.matmul(out=z1p[:], lhsT=hgT[:], rhs=Wc1_sb[:],
                             start=True, stop=True)
            z1 = sb_pool.tile([G, P], FP, tag="z1")
            nc.scalar.activation(z1[:], z1p[:], AFT.Relu)
            z1Tp = mini_ps.tile([P, G], FP, tag="mini")
            nc.tensor.transpose(z1Tp[:], z1[:], ident_f[:G, :G])
            z1T = sb_pool.tile([P, G], FP, tag="z1T")
            nc.scalar.copy(z1T[:], z1Tp[:])
            z2p = mini_ps.tile([G, 64], FP, tag="mini")
            nc.tensor.matmul(out=z2p[:], lhsT=z1T[:], rhs=Wc2_sb[:],
                             start=True, stop=True)
            z2 = sb_pool.tile([G, 64], FP, tag="z2")
            nc.scalar.activation(z2[:], z2p[:], AFT.Relu)
            z2Tp = mini_ps.tile([64, G], FP, tag="mini")
            nc.tensor.transpose(z2Tp[:], z2[:], ident_f[:G, :G])
            z2T = sb_pool.tile([64, G], FP, tag="z2T")
            nc.scalar.copy(z2T[:], z2Tp[:])
            z3p = mini_ps.tile([G, OUT], FP, tag="mini")
            nc.tensor.matmul(out=z3p[:], lhsT=z2T[:], rhs=Wc3_sb[:],
                             start=True, stop=True)
            z3 = sb_pool.tile([G, OUT], FP, tag="z3")
            nc.scalar.copy(z3[:], z3p[:])
            nc.sync.dma_start(out_ext[:], z3[:])

    nc.compile()
    return nc


# ---------------------------------------------------------------------------
# Entry point
# ---------------------------------------------------------------------------

def _run(inputs, nw_per_core=49, trace=False):
    from concourse.bass_utils import run_bass_kernel_spmd

    src = np.asarray(inputs["src"])
    dst = np.asarray(inputs["dst"])
    n2g = np.asarray(inputs["node2graph"])
    feat = np.asarray(inputs["feature"], np.float32)

    cfg, per_core, ns, nd, perm_src = prep(src, dst, n2g, nw_per_core)
    NP = cfg["NP"]

    featp = np.zeros((NP, P), np.float32)
    featp[: feat.shape[0]] = feat
    featp *= ns[:, None]
    table0 = np.ascontiguousarray(featp[perm_src]).astype(bf16)

    def b(x):
        return np.ascontiguousarray(np.asarray(x, np.float32).astype(bf16))

    SPLIT = cfg["SPLIT"]
    common = dict(
        table0lo=np.ascontiguousarray(table0[:SPLIT]),
        table0hi=np.ascontiguousarray(table0[SPLIT:]),
        Wgc0=b(inputs["W_gc1"]), Wgc1=b(inputs["W_gc2"]),
        Wc1=np.ascontiguousarray(np.asarray(inputs["Wc1"], np.float32)),
        Wc2=np.ascontiguousarray(np.asarray(inputs["Wc2"], np.float32)),
        Wc3=np.ascontiguousarray(np.asarray(inputs["Wc3"], np.float32)),
    )
    attn = np.asarray(inputs["attn"], np.float32)
    for i in range(3):
        common[f"Ws{i}"] = b(np.asarray(inputs["W_src"], np.float32)[i])
        common[f"Wd{i}"] = b(np.asarray(inputs["W_dst"], np.float32)[i])
        ar = np.zeros((P, HEADS), np.float32)
        for h in range(HEADS):
            ar[h * DH : (h + 1) * DH, h] = attn[i][h]
        common[f"abd{i}"] = np.ascontiguousarray(ar).astype(bf16)

    in_maps = []
    for c in range(N_CORES):
        m = dict(common)
        m.update(per_core[c])
        in_maps.append(m)

    nc = build_nc(cfg)
    res = run_bass_kernel_spmd(nc, in_maps, core_ids=list(range(N_CORES)),
                               trace=trace)
    return np.asarray(res.results[0]["out"], np.float32), res


def kernel(**inputs) -> np.ndarray:
    out, _ = _run(inputs)
    return out

